# revision 7
# baseline (speedup 1.0000x reference)
"""AttnDecoderRNN step on 8 TRN2 NeuronCores (Bass/Tile).

Sharding (per sharding hint): vocab-parallel out projection (embedding row
handled as a host-side gather/shard selection), hidden-sharded comb/GRU
matmuls, replicated attention. Collectives: AllGather of x (post-comb relu),
AllGather of h', AllGather of per-core log-softmax stats (max, sumexp).

Engine assignment (avoids the gpsimd/SWDGE backlog that delayed collective
triggers): gpsimd carries ONLY collective bounces + triggers + gathered
loads; small inputs ride the scalar HWDGE ring; big weights go first on the
sync ring ahead of the streamed out_W tiles.

Shapes: NHID=1024, NOUT=50257, MAX_LEN=24, batch=1.
Per-core vocab shard: VS=6400 (8*6400=51200 >= 50257; padding gets bias -1e4).
"""
import os
import sys
import types
import contextlib
import ctypes

import numpy as np

# ---------------------------------------------------------------------------
# antenv.axon_hooks shim: the container's antenv stub lacks this module, but
# concourse.bass_utils imports it when tracing is requested (BASS_TRACE=1).
# Provide it, with the ctypes NTFF profile hook libaxon exposes.
# ---------------------------------------------------------------------------
_HOOK = [None]


def _install_axon_hook_shim():
    if "antenv.axon_hooks" not in sys.modules:
        mod = types.ModuleType("antenv.axon_hooks")

        def set_axon_ntff_profile_hook(h):
            _HOOK[0] = h

        def get_axon_ntff_profile_hook():
            return _HOOK[0]

        mod.set_axon_ntff_profile_hook = set_axon_ntff_profile_hook
        mod.get_axon_ntff_profile_hook = get_axon_ntff_profile_hook
        sys.modules["antenv.axon_hooks"] = mod
        try:
            import antenv

            antenv.axon_hooks = mod
        except ImportError:
            pass
    if _HOOK[0] is None:
        so_path = "/opt/axon/libaxon_pjrt.so"
        try:
            lib = ctypes.CDLL(so_path)
        except OSError:
            return
        if not hasattr(lib, "axon_start_nrt_profile"):
            return
        lib.axon_start_nrt_profile.argtypes = [
            ctypes.POINTER(ctypes.c_int64),
            ctypes.c_size_t,
        ]
        lib.axon_start_nrt_profile.restype = ctypes.c_int64
        lib.axon_stop_nrt_profile.argtypes = [ctypes.c_char_p]
        lib.axon_stop_nrt_profile.restype = ctypes.c_int64

        @contextlib.contextmanager
        def _hook(output_dir, device_ids):
            import jax

            jax.devices()
            if device_ids:
                ids = (ctypes.c_int64 * len(device_ids))(*device_ids)
                rc = lib.axon_start_nrt_profile(ids, len(device_ids))
            else:
                rc = lib.axon_start_nrt_profile(None, 0)
            if rc != 0:
                raise RuntimeError(f"axon_start_nrt_profile rc={rc}")
            try:
                yield
            finally:
                n = lib.axon_stop_nrt_profile(str(output_dir).encode())
                print(f"profile: {n} file(s) -> {output_dir}", file=sys.stderr)

        sys.modules["antenv.axon_hooks"].set_axon_ntff_profile_hook(_hook)


_install_axon_hook_shim()

NCORES = 8
NHID = 1024
NOUT = 50257
MAX_LEN = 24
HC = NHID // 128          # 8 hidden chunks of 128
VS = 6400                 # vocab rows per core (padded)
TN = 400                  # out-projection free-dim tile (PSUM bank limit: 512 f32)
NT = VS // TN             # 16 logical tiles per core
WG = 4                    # logical tiles per W DMA chunk
PAD_BIAS = -1.0e4         # bias on padded vocab rows: exp() underflows to 0
WOUT_BF16 = os.environ.get("WOUT_DTYPE", "bf16") == "bf16"

_CACHE = {}


def _build():
    import concourse.bass as bass
    import concourse.tile as tile
    from concourse import bacc, mybir, masks
    from contextlib import ExitStack

    f32 = mybir.dt.float32
    wdt = mybir.dt.bfloat16 if WOUT_BF16 else f32

    nc = bacc.Bacc(
        "TRN2",
        target_bir_lowering=False,
        debug=False,
        enable_asserts=True,
        num_devices=NCORES,
    )

    # ---- I/O ----
    emb_in = nc.dram_tensor("emb_in", [128, HC], f32, kind="ExternalInput")
    h0c_in = nc.dram_tensor("h0c_in", [128, HC], f32, kind="ExternalInput")
    h0own_in = nc.dram_tensor("h0own_in", [128, 1], f32, kind="ExternalInput")
    enc_in = nc.dram_tensor("enc_in", [MAX_LEN, NHID], f32, kind="ExternalInput")
    attnw_in = nc.dram_tensor("attnw_in", [128, 16 * MAX_LEN], f32, kind="ExternalInput")
    attnb_in = nc.dram_tensor("attnb_in", [1, MAX_LEN], f32, kind="ExternalInput")
    combw_in = nc.dram_tensor("combw_in", [128, 16 * 128], f32, kind="ExternalInput")
    combb_in = nc.dram_tensor("combb_in", [128, 1], f32, kind="ExternalInput")
    wih_in = nc.dram_tensor("wih_in", [128, 3 * HC * 128], f32, kind="ExternalInput")
    whh_in = nc.dram_tensor("whh_in", [128, 3 * HC * 128], f32, kind="ExternalInput")
    bih_in = nc.dram_tensor("bih_in", [128, 3], f32, kind="ExternalInput")
    bhh_in = nc.dram_tensor("bhh_in", [128, 3], f32, kind="ExternalInput")
    wout_in = nc.dram_tensor(
        "wout_in", [NT // WG, 128, WG * HC * TN], wdt, kind="ExternalInput"
    )
    bout_in = nc.dram_tensor("bout_in", [16, TN], f32, kind="ExternalInput")

    logp_out = nc.dram_tensor("logp_out", [16, TN], f32, kind="ExternalOutput")
    h_out = nc.dram_tensor("h_out", [128, 1], f32, kind="ExternalOutput")
    attn_out = nc.dram_tensor("attn_out", [1, MAX_LEN], f32, kind="ExternalOutput")

    RG = [list(range(NCORES))]

    with tile.TileContext(nc) as tc:
        with ExitStack() as ctx:
            wpool = ctx.enter_context(tc.tile_pool(name="wpool", bufs=NT // WG))
            cpool = ctx.enter_context(tc.tile_pool(name="cpool", bufs=1))
            spool = ctx.enter_context(tc.tile_pool(name="spool", bufs=2))
            pp = ctx.enter_context(tc.tile_pool(name="pp", bufs=2, space="PSUM"))
            dram = ctx.enter_context(tc.tile_pool(name="dram", bufs=1, space="DRAM"))

            # ---- constants / small inputs to SBUF (scalar HWDGE ring) ----
            ident = cpool.tile([128, 128], f32)
            masks.make_identity(nc, ident[:])
            ones_row = cpool.tile([1, 128], f32)   # [1,P] lhsT for broadcasts
            nc.gpsimd.memset(ones_row[:], 1.0)
            ones_col = cpool.tile([128, 1], f32)   # [P,1] rhs for partition sums
            nc.gpsimd.memset(ones_col[:], 1.0)

            emb_sb = cpool.tile([128, HC], f32)
            nc.scalar.dma_start(emb_sb[:], emb_in[:])
            h0c_sb = cpool.tile([128, HC], f32)
            nc.scalar.dma_start(h0c_sb[:], h0c_in[:])
            h0own_sb = cpool.tile([128, 1], f32)
            nc.scalar.dma_start(h0own_sb[:], h0own_in[:])
            enc_sb = cpool.tile([MAX_LEN, NHID], f32)
            nc.scalar.dma_start(enc_sb[:], enc_in[:])
            attnw_sb = cpool.tile([128, 16 * MAX_LEN], f32)
            nc.scalar.dma_start(attnw_sb[:], attnw_in[:])
            attnb_sb = cpool.tile([1, MAX_LEN], f32)
            nc.scalar.dma_start(attnb_sb[:], attnb_in[:])
            combb_sb = cpool.tile([128, 1], f32)
            nc.scalar.dma_start(combb_sb[:], combb_in[:])
            bih_sb = cpool.tile([128, 3], f32)
            nc.scalar.dma_start(bih_sb[:], bih_in[:])
            bhh_sb = cpool.tile([128, 3], f32)
            nc.scalar.dma_start(bhh_sb[:], bhh_in[:])
            bout_sb = cpool.tile([16, TN], f32)
            nc.scalar.dma_start(bout_sb[:], bout_in[:])

            # big chain weights on the sync ring, AHEAD of the W stream
            combw_sb = cpool.tile([128, 16 * 128], f32)
            nc.sync.dma_start(combw_sb[:], combw_in[:])
            wih_sb = cpool.tile([128, 3 * HC * 128], f32)
            nc.sync.dma_start(wih_sb[:], wih_in[:])
            whh_sb = cpool.tile([128, 3 * HC * 128], f32)
            nc.sync.dma_start(whh_sb[:], whh_in[:])

            # ---- attention (replicated) ----
            psA = pp.tile([128, MAX_LEN], f32, tag="psA", bufs=1)
            alog_ps = psA[0:1, 0:MAX_LEN]
            for c in range(HC):
                nc.tensor.matmul(
                    alog_ps,
                    emb_sb[:, c : c + 1],
                    attnw_sb[:, c * MAX_LEN : (c + 1) * MAX_LEN],
                    start=(c == 0),
                    stop=False,
                )
            for c in range(HC):
                nc.tensor.matmul(
                    alog_ps,
                    h0c_sb[:, c : c + 1],
                    attnw_sb[:, (HC + c) * MAX_LEN : (HC + c + 1) * MAX_LEN],
                    start=False,
                    stop=(c == HC - 1),
                )
            alog_sb = spool.tile([1, MAX_LEN], f32)
            nc.vector.tensor_add(alog_sb[:], alog_ps, attnb_sb[:])
            amax = spool.tile([1, 1], f32)
            nc.vector.reduce_max(amax[:], alog_sb[:], axis=mybir.AxisListType.X)
            namax = spool.tile([1, 1], f32)
            nc.vector.tensor_scalar_mul(namax[:], amax[:], -1.0)
            probs = spool.tile([1, MAX_LEN], f32)
            sume = spool.tile([1, 1], f32)
            nc.scalar.activation(
                probs[:], alog_sb[:], mybir.ActivationFunctionType.Exp,
                bias=namax[0:1, 0:1], accum_out=sume[:],
            )
            rinv = spool.tile([1, 1], f32)
            nc.vector.reciprocal(rinv[:], sume[:])
            attnp_sb = spool.tile([1, MAX_LEN], f32)
            nc.scalar.mul(attnp_sb[:], probs[:], rinv[0:1, 0:1])
            nc.scalar.dma_start(attn_out[:], attnp_sb[:])

            # transpose attn probs -> [24, 1]
            psS = pp.tile([128, 16], f32, tag="psS", bufs=2)
            nc.tensor.transpose(psS[0:MAX_LEN, 0:1], attnp_sb[:], ident[0:1, 0:1])
            awt_sb = spool.tile([MAX_LEN, 1], f32)
            nc.vector.tensor_copy(awt_sb[:], psS[0:MAX_LEN, 0:1])

            # attn_applied chunks: [128, HC]
            psA2 = pp.tile([128, MAX_LEN], f32, tag="psA", bufs=1)
            for c in range(HC):
                nc.tensor.matmul(
                    psA2[:, c : c + 1],
                    enc_sb[0:MAX_LEN, c * 128 : (c + 1) * 128],
                    awt_sb[:],
                    start=True,
                    stop=True,
                )
            aap_sb = spool.tile([128, HC], f32)
            nc.vector.tensor_copy(aap_sb[:], psA2[:, 0:HC])

            # ---- comb (sharded): x_shard = relu(cat(emb, aap) @ comb_W_sh.T + b) ----
            psC = pp.tile([128, 1], f32, tag="psG", bufs=2)
            for c in range(HC):
                nc.tensor.matmul(
                    psC[:],
                    combw_sb[:, c * 128 : (c + 1) * 128],
                    emb_sb[:, c : c + 1],
                    start=(c == 0),
                    stop=False,
                )
            for c in range(HC):
                nc.tensor.matmul(
                    psC[:],
                    combw_sb[:, (HC + c) * 128 : (HC + c + 1) * 128],
                    aap_sb[:, c : c + 1],
                    start=False,
                    stop=(c == HC - 1),
                )
            xsh_sb = spool.tile([128, 1], f32)
            nc.scalar.activation(
                xsh_sb[:], psC[:], mybir.ActivationFunctionType.Relu,
                bias=combb_sb[:, 0:1],
            )

            # ---- AllGather x (gpsimd carries only bounces + triggers) ----
            xa_in = dram.tile([128, 1], f32)
            xa_out = dram.tile([NHID, 1], f32, addr_space="Shared")
            nc.gpsimd.dma_start(xa_in[:], xsh_sb[:])
            nc.gpsimd.collective_compute(
                "AllGather", mybir.AluOpType.bypass, replica_groups=RG,
                ins=[xa_in[:].opt()], outs=[xa_out[:].opt()],
            )
            x_sb = cpool.tile([128, HC], f32)
            nc.gpsimd.dma_start(
                x_sb[:], xa_out[:].rearrange("(c p) o -> p (c o)", p=128)
            )

            # ---- GRU (sharded) ----
            # r and z: gi + gh fused in one accumulation group
            psR = pp.tile([128, 1], f32, tag="psG", bufs=2)
            for k in range(HC):
                nc.tensor.matmul(
                    psR[:], wih_sb[:, (0 * HC + k) * 128 : (0 * HC + k + 1) * 128],
                    x_sb[:, k : k + 1], start=(k == 0), stop=False)
            for k in range(HC):
                nc.tensor.matmul(
                    psR[:], whh_sb[:, (0 * HC + k) * 128 : (0 * HC + k + 1) * 128],
                    h0c_sb[:, k : k + 1], start=False, stop=(k == HC - 1))
            brz_sb = spool.tile([128, 2], f32)
            nc.vector.tensor_add(brz_sb[:], bih_sb[:, 0:2], bhh_sb[:, 0:2])
            r_sb = spool.tile([128, 1], f32)
            nc.scalar.activation(
                r_sb[:], psR[:], mybir.ActivationFunctionType.Sigmoid,
                bias=brz_sb[:, 0:1])

            psZ = pp.tile([128, 1], f32, tag="psG", bufs=2)
            for k in range(HC):
                nc.tensor.matmul(
                    psZ[:], wih_sb[:, (1 * HC + k) * 128 : (1 * HC + k + 1) * 128],
                    x_sb[:, k : k + 1], start=(k == 0), stop=False)
            for k in range(HC):
                nc.tensor.matmul(
                    psZ[:], whh_sb[:, (1 * HC + k) * 128 : (1 * HC + k + 1) * 128],
                    h0c_sb[:, k : k + 1], start=False, stop=(k == HC - 1))
            z_sb = spool.tile([128, 1], f32)
            nc.scalar.activation(
                z_sb[:], psZ[:], mybir.ActivationFunctionType.Sigmoid,
                bias=brz_sb[:, 1:2])

            psIN = pp.tile([128, 1], f32, tag="psG", bufs=2)
            for k in range(HC):
                nc.tensor.matmul(
                    psIN[:], wih_sb[:, (2 * HC + k) * 128 : (2 * HC + k + 1) * 128],
                    x_sb[:, k : k + 1], start=(k == 0), stop=(k == HC - 1))
            psHN = pp.tile([128, 1], f32, tag="psG", bufs=2)
            for k in range(HC):
                nc.tensor.matmul(
                    psHN[:], whh_sb[:, (2 * HC + k) * 128 : (2 * HC + k + 1) * 128],
                    h0c_sb[:, k : k + 1], start=(k == 0), stop=(k == HC - 1))

            hnb_sb = spool.tile([128, 1], f32)
            nc.scalar.activation(
                hnb_sb[:], psHN[:], mybir.ActivationFunctionType.Identity,
                bias=bhh_sb[:, 2:3])
            rhn_sb = spool.tile([128, 1], f32)
            nc.vector.tensor_mul(rhn_sb[:], r_sb[:], hnb_sb[:])
            t1_sb = spool.tile([128, 1], f32)
            nc.vector.tensor_add(t1_sb[:], psIN[:], rhn_sb[:])
            n_sb = spool.tile([128, 1], f32)
            nc.scalar.activation(
                n_sb[:], t1_sb[:], mybir.ActivationFunctionType.Tanh,
                bias=bih_sb[:, 2:3])
            d_sb = spool.tile([128, 1], f32)
            nc.vector.tensor_sub(d_sb[:], h0own_sb[:], n_sb[:])
            zd_sb = spool.tile([128, 1], f32)
            nc.vector.tensor_mul(zd_sb[:], z_sb[:], d_sb[:])
            hn_sb = spool.tile([128, 1], f32)
            nc.vector.tensor_add(hn_sb[:], n_sb[:], zd_sb[:])
            nc.scalar.dma_start(h_out[:], hn_sb[:])

            # ---- AllGather h' ----
            ha_in = dram.tile([128, 1], f32)
            ha_out = dram.tile([NHID, 1], f32, addr_space="Shared")
            nc.gpsimd.dma_start(ha_in[:], hn_sb[:])
            nc.gpsimd.collective_compute(
                "AllGather", mybir.AluOpType.bypass, replica_groups=RG,
                ins=[ha_in[:].opt()], outs=[ha_out[:].opt()],
            )
            h_sb = cpool.tile([128, HC], f32)
            nc.gpsimd.dma_start(
                h_sb[:], ha_out[:].rearrange("(c p) o -> p (c o)", p=128)
            )
            h_mm = cpool.tile([128, HC], wdt)
            nc.vector.tensor_copy(h_mm[:], h_sb[:])

            # ---- out projection (streamed, vocab shard VS=6400, 16 tiles) ----
            logits0_sb = cpool.tile([16, TN], f32)
            for wc in range(NT // WG):
                w_tile = wpool.tile([128, WG * HC * TN], wdt, tag="wtile")
                nc.sync.dma_start(w_tile[:], wout_in[wc])
                for ti in range(WG):
                    t = wc * WG + ti
                    psT = pp.tile([1, TN], f32, tag="psT", bufs=2)
                    for k in range(HC):
                        nc.tensor.matmul(
                            psT[:],
                            h_mm[:, k : k + 1],
                            w_tile[:, (ti * HC + k) * TN : (ti * HC + k + 1) * TN],
                            start=(k == 0),
                            stop=(k == HC - 1),
                        )
                    # compute engines can't address partition t directly (32-part
                    # alignment) — stage on partition 0, DMA-scatter to row t
                    lrow = spool.tile([1, TN], f32, tag="lrow", bufs=3)
                    nc.vector.tensor_copy(lrow[:], psT[:])
                    nc.scalar.dma_start(logits0_sb[t : t + 1, :], lrow[:])
            # bias add (also applies the -1e4 padding bias)
            logits_sb = cpool.tile([16, TN], f32)
            nc.vector.tensor_add(logits_sb[:], logits0_sb[:], bout_sb[:])

            # ---- local log-softmax stats ----
            mx16 = spool.tile([16, 1], f32)
            nc.vector.reduce_max(mx16[:], logits_sb[:], axis=mybir.AxisListType.X)
            psS2 = pp.tile([128, 16], f32, tag="psS", bufs=2)
            nc.tensor.transpose(psS2[0:1, 0:16], mx16[:], ident[0:16, 0:16])
            mt_sb = spool.tile([1, 16], f32)
            nc.vector.tensor_copy(mt_sb[:], psS2[0:1, 0:16])
            mc = spool.tile([1, 1], f32)
            nc.vector.reduce_max(mc[:], mt_sb[:], axis=mybir.AxisListType.X)
            nmc = spool.tile([1, 1], f32)
            nc.vector.tensor_scalar_mul(nmc[:], mc[:], -1.0)
            psB = pp.tile([16, 1], f32, tag="psS", bufs=2)
            nc.tensor.matmul(psB[:], ones_row[0:1, 0:16], nmc[:], start=True, stop=True)
            nm16_sb = spool.tile([16, 1], f32)
            nc.vector.tensor_copy(nm16_sb[:], psB[:])
            e16 = spool.tile([16, TN], f32)
            zrow = spool.tile([16, 1], f32)
            nc.scalar.activation(
                e16[:], logits_sb[:], mybir.ActivationFunctionType.Exp,
                bias=nm16_sb[:, 0:1], accum_out=zrow[:],
            )
            psZc = pp.tile([1, 1], f32, tag="psS", bufs=2)
            nc.tensor.matmul(psZc[:], zrow[:], ones_col[0:16, 0:1], start=True, stop=True)
            stats_sb = spool.tile([1, 2], f32)
            nc.vector.tensor_copy(stats_sb[0:1, 0:1], mc[:])
            nc.vector.tensor_copy(stats_sb[0:1, 1:2], psZc[:])

            # ---- AllGather stats ----
            st_in = dram.tile([1, 2], f32)
            st_out = dram.tile([NCORES, 2], f32, addr_space="Shared")
            nc.gpsimd.dma_start(st_in[:], stats_sb[:])
            nc.gpsimd.collective_compute(
                "AllGather", mybir.AluOpType.bypass, replica_groups=RG,
                ins=[st_in[:].opt()], outs=[st_out[:].opt()],
            )
            s8_sb = spool.tile([NCORES, 2], f32)
            nc.gpsimd.dma_start(s8_sb[:], st_out[:])

            # global max M, then C = M + ln(sum_c Z_c exp(m_c - M))
            psM = pp.tile([128, 16], f32, tag="psS", bufs=2)
            nc.tensor.transpose(psM[0:1, 0:NCORES], s8_sb[:, 0:1], ident[0:NCORES, 0:NCORES])
            m1_sb = spool.tile([1, NCORES], f32)
            nc.vector.tensor_copy(m1_sb[:], psM[0:1, 0:NCORES])
            gM = spool.tile([1, 1], f32)
            nc.vector.reduce_max(gM[:], m1_sb[:], axis=mybir.AxisListType.X)
            ngM = spool.tile([1, 1], f32)
            nc.vector.tensor_scalar_mul(ngM[:], gM[:], -1.0)
            psB2 = pp.tile([NCORES, 1], f32, tag="psS", bufs=2)
            nc.tensor.matmul(psB2[:], ones_row[0:1, 0:NCORES], ngM[:], start=True, stop=True)
            ngM8_sb = spool.tile([NCORES, 1], f32)
            nc.vector.tensor_copy(ngM8_sb[:], psB2[:])
            e8 = spool.tile([NCORES, 1], f32)
            nc.scalar.activation(
                e8[:], s8_sb[:, 0:1], mybir.ActivationFunctionType.Exp,
                bias=ngM8_sb[:, 0:1])
            s8p = spool.tile([NCORES, 1], f32)
            nc.vector.tensor_mul(s8p[:], e8[:], s8_sb[:, 1:2])
            psZg = pp.tile([1, 1], f32, tag="psS", bufs=2)
            nc.tensor.matmul(psZg[:], s8p[:], ones_col[0:NCORES, 0:1], start=True, stop=True)
            lnZ = spool.tile([1, 1], f32)
            nc.scalar.activation(lnZ[:], psZg[:], mybir.ActivationFunctionType.Ln)
            C = spool.tile([1, 1], f32)
            nc.scalar.activation(
                C[:], lnZ[:], mybir.ActivationFunctionType.Identity,
                bias=gM[0:1, 0:1])
            nC = spool.tile([1, 1], f32)
            nc.vector.tensor_scalar_mul(nC[:], C[:], -1.0)
            psB3 = pp.tile([16, 1], f32, tag="psS", bufs=2)
            nc.tensor.matmul(psB3[:], ones_row[0:1, 0:16], nC[:], start=True, stop=True)
            nC16_sb = spool.tile([16, 1], f32)
            nc.vector.tensor_copy(nC16_sb[:], psB3[:])
            logp_sb = spool.tile([16, TN], f32)
            nc.scalar.activation(
                logp_sb[:], logits_sb[:], mybir.ActivationFunctionType.Identity,
                bias=nC16_sb[:, 0:1])
            nc.sync.dma_start(logp_out[:], logp_sb[:])

    nc.compile()
    return nc


def _prep_inputs(inp, hidden, encoder_outputs, emb_W, attn_W, attn_b,
                 comb_W, comb_b, W_ih, W_hh, b_ih, b_hh, out_W, out_b):
    """Shard/layout the full inputs into 8 per-core input maps."""
    f = np.float32
    idx = int(np.asarray(inp).ravel()[0])
    emb_row = np.asarray(emb_W[idx], dtype=f)                 # [1024]
    h0 = np.asarray(hidden, dtype=f).ravel()                  # [1024]
    enc = np.ascontiguousarray(np.asarray(encoder_outputs, dtype=f))  # [24,1024]

    def chunked_vec(v):
        # [1024] -> [128, 8] with [p, c] = v[c*128+p]
        return np.ascontiguousarray(v.reshape(HC, 128).T)

    emb_c = chunked_vec(emb_row)
    h0_c = chunked_vec(h0)

    # attn_W [24, 2048] -> [128, 16*24]
    aT = np.asarray(attn_W, dtype=f).T.reshape(16, 128, MAX_LEN)
    attnw = np.ascontiguousarray(aT.transpose(1, 0, 2).reshape(128, 16 * MAX_LEN))
    attnb = np.ascontiguousarray(np.asarray(attn_b, dtype=f).reshape(1, MAX_LEN))

    comb_W = np.asarray(comb_W, dtype=f)
    comb_b_a = np.asarray(comb_b, dtype=f)
    W_ih_a = np.asarray(W_ih, dtype=f)
    W_hh_a = np.asarray(W_hh, dtype=f)
    b_ih_a = np.asarray(b_ih, dtype=f)
    b_hh_a = np.asarray(b_hh, dtype=f)
    out_W_a = np.asarray(out_W, dtype=f)
    out_b_a = np.asarray(out_b, dtype=f)

    if WOUT_BF16:
        import ml_dtypes

        wout_dt = ml_dtypes.bfloat16
    else:
        wout_dt = f

    in_maps = []
    for j in range(NCORES):
        sl = slice(j * 128, (j + 1) * 128)
        # comb shard [128, 2048] -> [128(p), 16*128]
        cw = comb_W[sl]                                    # [128, 2048]
        cwT = cw.T.reshape(16, 128, 128)                   # [c, p, m]
        combw = np.ascontiguousarray(cwT.transpose(1, 0, 2).reshape(128, 16 * 128))
        combb = np.ascontiguousarray(comb_b_a[sl].reshape(128, 1))

        def gate_pack(W):
            cols = []
            for g in range(3):
                Wg = W[g * NHID + j * 128 : g * NHID + (j + 1) * 128]  # [128, 1024]
                T = Wg.T.reshape(HC, 128, 128)                          # [k, p, m]
                cols.append(T.transpose(1, 0, 2).reshape(128, HC * 128))
            return np.ascontiguousarray(np.concatenate(cols, axis=1))

        wih = gate_pack(W_ih_a)
        whh = gate_pack(W_hh_a)
        bih = np.ascontiguousarray(
            np.stack([b_ih_a[g * NHID + j * 128 : g * NHID + (j + 1) * 128]
                      for g in range(3)], axis=1))
        bhh = np.ascontiguousarray(
            np.stack([b_hh_a[g * NHID + j * 128 : g * NHID + (j + 1) * 128]
                      for g in range(3)], axis=1))

        # out_W vocab shard [VS, 1024] (zero-padded), bias shard with PAD_BIAS
        lo, hi = j * VS, min((j + 1) * VS, NOUT)
        nreal = max(0, hi - lo)
        wsh = np.zeros((VS, NHID), dtype=f)
        bsh = np.full((VS,), PAD_BIAS, dtype=f)
        if nreal > 0:
            wsh[:nreal] = out_W_a[lo:hi]
            bsh[:nreal] = out_b_a[lo:hi]
        WT = wsh.T                                         # [1024, 6400]
        # [NT, 128, HC*TN]: [t, p, k*TN+n] = WT[k*128+p, t*TN+n], grouped by WG
        warr = (
            WT.reshape(HC, 128, NT, TN).transpose(2, 1, 0, 3)
            .reshape(NT // WG, WG, 128, HC * TN).transpose(0, 2, 1, 3)
            .reshape(NT // WG, 128, WG * HC * TN)
        )
        warr = np.ascontiguousarray(warr.astype(wout_dt))
        barr = np.ascontiguousarray(bsh.reshape(16, TN))

        in_maps.append({
            "emb_in": emb_c, "h0c_in": h0_c,
            "h0own_in": np.ascontiguousarray(h0[sl].reshape(128, 1)),
            "enc_in": enc, "attnw_in": attnw, "attnb_in": attnb,
            "combw_in": combw, "combb_in": combb,
            "wih_in": wih, "whh_in": whh, "bih_in": bih, "bhh_in": bhh,
            "wout_in": warr, "bout_in": barr,
        })
    return in_maps


def run(trace=False, **inputs):
    from concourse.bass_utils import run_bass_kernel_spmd

    if "nc" not in _CACHE:
        _CACHE["nc"] = _build()
    nc = _CACHE["nc"]

    inputs.pop("encoder_output", None)  # unused by the reference computation
    in_maps = _prep_inputs(**inputs)
    res = run_bass_kernel_spmd(
        nc, in_maps, core_ids=list(range(NCORES)), trace=trace
    )

    logp = np.concatenate(
        [res.results[j]["logp_out"].reshape(-1) for j in range(NCORES)]
    )[:NOUT].reshape(1, NOUT).astype(np.float32)
    h = np.concatenate(
        [res.results[j]["h_out"].reshape(-1) for j in range(NCORES)]
    ).reshape(1, 1, NHID).astype(np.float32)
    attn = res.results[0]["attn_out"].reshape(1, MAX_LEN).astype(np.float32)
    return (logp, h, attn), res


def kernel(**inputs):
    out, _ = run(trace=bool(os.environ.get("KERNEL_TRACE")), **inputs)
    return out


# revision 13
# speedup vs baseline: 1.6324x; 1.6324x over previous
"""AttnDecoderRNN step on 8 TRN2 NeuronCores (Bass/Tile).

Sharding (per sharding hint): vocab-parallel out projection (embedding row
handled as a host-side gather/shard selection), hidden-sharded comb/GRU
matmuls, replicated attention. Collectives: AllGather of x (post-comb relu),
AllGather of h', AllGather of per-core log-softmax stats (max, sumexp).

Engine assignment (avoids the gpsimd/SWDGE backlog that delayed collective
triggers): gpsimd carries ONLY collective bounces + triggers + gathered
loads; small inputs ride the scalar HWDGE ring; big weights go first on the
sync ring ahead of the streamed out_W tiles.

Shapes: NHID=1024, NOUT=50257, MAX_LEN=24, batch=1.
Per-core vocab shard: VS=6400 (8*6400=51200 >= 50257; padding gets bias -1e4).
"""
import os
import sys
import types
import contextlib
import ctypes

import numpy as np

# ---------------------------------------------------------------------------
# antenv.axon_hooks shim: the container's antenv stub lacks this module, but
# concourse.bass_utils imports it when tracing is requested (BASS_TRACE=1).
# Provide it, with the ctypes NTFF profile hook libaxon exposes.
# ---------------------------------------------------------------------------
_HOOK = [None]


def _install_axon_hook_shim():
    if "antenv.axon_hooks" not in sys.modules:
        mod = types.ModuleType("antenv.axon_hooks")

        def set_axon_ntff_profile_hook(h):
            _HOOK[0] = h

        def get_axon_ntff_profile_hook():
            return _HOOK[0]

        mod.set_axon_ntff_profile_hook = set_axon_ntff_profile_hook
        mod.get_axon_ntff_profile_hook = get_axon_ntff_profile_hook
        sys.modules["antenv.axon_hooks"] = mod
        try:
            import antenv

            antenv.axon_hooks = mod
        except ImportError:
            pass
    if _HOOK[0] is None:
        so_path = "/opt/axon/libaxon_pjrt.so"
        try:
            lib = ctypes.CDLL(so_path)
        except OSError:
            return
        if not hasattr(lib, "axon_start_nrt_profile"):
            return
        lib.axon_start_nrt_profile.argtypes = [
            ctypes.POINTER(ctypes.c_int64),
            ctypes.c_size_t,
        ]
        lib.axon_start_nrt_profile.restype = ctypes.c_int64
        lib.axon_stop_nrt_profile.argtypes = [ctypes.c_char_p]
        lib.axon_stop_nrt_profile.restype = ctypes.c_int64

        @contextlib.contextmanager
        def _hook(output_dir, device_ids):
            import jax

            jax.devices()
            if device_ids:
                ids = (ctypes.c_int64 * len(device_ids))(*device_ids)
                rc = lib.axon_start_nrt_profile(ids, len(device_ids))
            else:
                rc = lib.axon_start_nrt_profile(None, 0)
            if rc != 0:
                raise RuntimeError(f"axon_start_nrt_profile rc={rc}")
            try:
                yield
            finally:
                n = lib.axon_stop_nrt_profile(str(output_dir).encode())
                print(f"profile: {n} file(s) -> {output_dir}", file=sys.stderr)

        sys.modules["antenv.axon_hooks"].set_axon_ntff_profile_hook(_hook)


_install_axon_hook_shim()

NCORES = 8
NHID = 1024
NOUT = 50257
MAX_LEN = 24
HC = NHID // 128          # 8 hidden chunks of 128
VS = 6400                 # vocab rows per core (padded)
TN = 400                  # out-projection free-dim tile (PSUM bank limit: 512 f32)
NT = VS // TN             # 16 logical tiles per core
WG = 4                    # logical tiles per W DMA chunk
PAD_BIAS = -1.0e4         # bias on padded vocab rows: exp() underflows to 0
WOUT_BF16 = os.environ.get("WOUT_DTYPE", "bf16") == "bf16"

_CACHE = {}


def _build():
    import concourse.bass as bass
    import concourse.tile as tile
    from concourse import bacc, mybir, masks
    from contextlib import ExitStack

    f32 = mybir.dt.float32
    wdt = mybir.dt.bfloat16 if WOUT_BF16 else f32

    nc = bacc.Bacc(
        "TRN2",
        target_bir_lowering=False,
        debug=False,
        enable_asserts=True,
        num_devices=NCORES,
    )

    # ---- I/O ----
    emb_in = nc.dram_tensor("emb_in", [128, HC], f32, kind="ExternalInput")
    h0c_in = nc.dram_tensor("h0c_in", [128, HC], f32, kind="ExternalInput")
    h0own_in = nc.dram_tensor("h0own_in", [128, 1], f32, kind="ExternalInput")
    enc_in = nc.dram_tensor("enc_in", [MAX_LEN, NHID], f32, kind="ExternalInput")
    attnw_in = nc.dram_tensor("attnw_in", [128, 16 * MAX_LEN], f32, kind="ExternalInput")
    attnb_in = nc.dram_tensor("attnb_in", [1, MAX_LEN], f32, kind="ExternalInput")
    combw_in = nc.dram_tensor("combw_in", [128, 16 * 128], f32, kind="ExternalInput")
    combb_in = nc.dram_tensor("combb_in", [128, 1], f32, kind="ExternalInput")
    wih_in = nc.dram_tensor("wih_in", [128, 3 * HC * 128], f32, kind="ExternalInput")
    whh_in = nc.dram_tensor("whh_in", [128, 3 * HC * 128], f32, kind="ExternalInput")
    bih_in = nc.dram_tensor("bih_in", [128, 3 * HC], f32, kind="ExternalInput")
    bhh_in = nc.dram_tensor("bhh_in", [128, 3 * HC], f32, kind="ExternalInput")
    wout_in = nc.dram_tensor(
        "wout_in", [NT // WG, 128, WG * HC * TN], wdt, kind="ExternalInput"
    )
    bout_in = nc.dram_tensor("bout_in", [16, TN], f32, kind="ExternalInput")

    logp_out = nc.dram_tensor("logp_out", [16, TN], f32, kind="ExternalOutput")
    h_out = nc.dram_tensor("h_out", [128, HC], f32, kind="ExternalOutput")
    attn_out = nc.dram_tensor("attn_out", [1, MAX_LEN], f32, kind="ExternalOutput")

    RG = [list(range(NCORES))]

    with tile.TileContext(nc) as tc:
        with ExitStack() as ctx:
            wpool = ctx.enter_context(tc.tile_pool(name="wpool", bufs=NT // WG))
            cpool = ctx.enter_context(tc.tile_pool(name="cpool", bufs=1))
            spool = ctx.enter_context(tc.tile_pool(name="spool", bufs=2))
            pp = ctx.enter_context(tc.tile_pool(name="pp", bufs=2, space="PSUM"))
            dram = ctx.enter_context(tc.tile_pool(name="dram", bufs=1, space="DRAM"))

            # ---- constants / small inputs to SBUF (scalar HWDGE ring) ----
            ident = cpool.tile([128, 128], f32)
            masks.make_identity(nc, ident[:])
            ones_row = cpool.tile([1, 128], f32)   # [1,P] lhsT for broadcasts
            nc.gpsimd.memset(ones_row[:], 1.0)
            ones_col = cpool.tile([128, 1], f32)   # [P,1] rhs for partition sums
            nc.gpsimd.memset(ones_col[:], 1.0)

            emb_sb = cpool.tile([128, HC], f32)
            nc.scalar.dma_start(emb_sb[:], emb_in[:])
            h0c_sb = cpool.tile([128, HC], f32)
            nc.scalar.dma_start(h0c_sb[:], h0c_in[:])
            h0own_sb = cpool.tile([128, 1], f32)
            nc.scalar.dma_start(h0own_sb[:], h0own_in[:])
            enc_sb = cpool.tile([MAX_LEN, NHID], f32)
            nc.scalar.dma_start(enc_sb[:], enc_in[:])
            attnw_sb = cpool.tile([128, 16 * MAX_LEN], f32)
            nc.scalar.dma_start(attnw_sb[:], attnw_in[:])
            attnb_sb = cpool.tile([1, MAX_LEN], f32)
            nc.scalar.dma_start(attnb_sb[:], attnb_in[:])
            combb_sb = cpool.tile([128, 1], f32)
            nc.scalar.dma_start(combb_sb[:], combb_in[:])
            bih_sb = cpool.tile([128, 3 * HC], f32)
            nc.scalar.dma_start(bih_sb[:], bih_in[:])
            bhh_sb = cpool.tile([128, 3 * HC], f32)
            nc.scalar.dma_start(bhh_sb[:], bhh_in[:])
            bout_sb = cpool.tile([16, TN], f32)
            nc.scalar.dma_start(bout_sb[:], bout_in[:])

            # chain weights also on the scalar ring so the sync ring is
            # dedicated to the streamed out_W tiles from t=0
            combw_sb = cpool.tile([128, 16 * 128], f32)
            nc.scalar.dma_start(combw_sb[:], combw_in[:])
            wih_sb = cpool.tile([128, 3 * HC * 128], f32)
            nc.scalar.dma_start(wih_sb[:], wih_in[:])
            whh_sb = cpool.tile([128, 3 * HC * 128], f32)
            nc.scalar.dma_start(whh_sb[:], whh_in[:])

            # ---- attention (replicated) ----
            psA = pp.tile([128, MAX_LEN], f32, tag="psA", bufs=1)
            alog_ps = psA[0:1, 0:MAX_LEN]
            for c in range(HC):
                nc.tensor.matmul(
                    alog_ps,
                    emb_sb[:, c : c + 1],
                    attnw_sb[:, c * MAX_LEN : (c + 1) * MAX_LEN],
                    start=(c == 0),
                    stop=False,
                )
            for c in range(HC):
                nc.tensor.matmul(
                    alog_ps,
                    h0c_sb[:, c : c + 1],
                    attnw_sb[:, (HC + c) * MAX_LEN : (HC + c + 1) * MAX_LEN],
                    start=False,
                    stop=(c == HC - 1),
                )
            alog_sb = spool.tile([1, MAX_LEN], f32)
            nc.vector.tensor_add(alog_sb[:], alog_ps, attnb_sb[:])
            amax = spool.tile([1, 1], f32)
            nc.vector.reduce_max(amax[:], alog_sb[:], axis=mybir.AxisListType.X)
            namax = spool.tile([1, 1], f32)
            nc.vector.tensor_scalar_mul(namax[:], amax[:], -1.0)
            probs = spool.tile([1, MAX_LEN], f32)
            sume = spool.tile([1, 1], f32)
            nc.scalar.activation(
                probs[:], alog_sb[:], mybir.ActivationFunctionType.Exp,
                bias=namax[0:1, 0:1], accum_out=sume[:],
            )
            rinv = spool.tile([1, 1], f32)
            nc.vector.reciprocal(rinv[:], sume[:])
            attnp_sb = spool.tile([1, MAX_LEN], f32)
            nc.scalar.mul(attnp_sb[:], probs[:], rinv[0:1, 0:1])
            nc.scalar.dma_start(attn_out[:], attnp_sb[:])

            # transpose attn probs -> [24, 1]
            psS = pp.tile([128, 16], f32, tag="psS", bufs=2)
            nc.tensor.transpose(psS[0:MAX_LEN, 0:1], attnp_sb[:], ident[0:1, 0:1])
            awt_sb = spool.tile([MAX_LEN, 1], f32)
            nc.vector.tensor_copy(awt_sb[:], psS[0:MAX_LEN, 0:1])

            # attn_applied chunks: [128, HC]
            psA2 = pp.tile([128, MAX_LEN], f32, tag="psA", bufs=1)
            for c in range(HC):
                nc.tensor.matmul(
                    psA2[:, c : c + 1],
                    enc_sb[0:MAX_LEN, c * 128 : (c + 1) * 128],
                    awt_sb[:],
                    start=True,
                    stop=True,
                )
            aap_sb = spool.tile([128, HC], f32)
            nc.vector.tensor_copy(aap_sb[:], psA2[:, 0:HC])

            # ---- comb (sharded): x_shard = relu(cat(emb, aap) @ comb_W_sh.T + b) ----
            psC = pp.tile([128, 1], f32, tag="psG", bufs=2)
            for c in range(HC):
                nc.tensor.matmul(
                    psC[:],
                    combw_sb[:, c * 128 : (c + 1) * 128],
                    emb_sb[:, c : c + 1],
                    start=(c == 0),
                    stop=False,
                )
            for c in range(HC):
                nc.tensor.matmul(
                    psC[:],
                    combw_sb[:, (HC + c) * 128 : (HC + c + 1) * 128],
                    aap_sb[:, c : c + 1],
                    start=False,
                    stop=(c == HC - 1),
                )
            xsh_sb = spool.tile([128, 1], f32)
            nc.scalar.activation(
                xsh_sb[:], psC[:], mybir.ActivationFunctionType.Relu,
                bias=combb_sb[:, 0:1],
            )

            # ---- GRU, contraction-sharded: core j contributes the partial
            # gate preactivations from its own x/h chunk; one AllReduce(add)
            # of [128, 32] then gives every core the full gate sums, and the
            # elementwise GRU yields the FULL h' replicated (no h gather).
            # Payload: psGA = gi+gh for r (cols 0:8) and z (8:16);
            #          psGB = i_n (0:8) and h_n (8:16) kept separate.
            psGA = pp.tile([128, 2 * HC], f32, tag="psG", bufs=2)
            for g in range(2):
                for c in range(HC):
                    col = g * HC + c
                    nc.tensor.matmul(
                        psGA[:, col : col + 1],
                        wih_sb[:, (g * HC + c) * 128 : (g * HC + c + 1) * 128],
                        xsh_sb[:], start=True, stop=False)
                    nc.tensor.matmul(
                        psGA[:, col : col + 1],
                        whh_sb[:, (g * HC + c) * 128 : (g * HC + c + 1) * 128],
                        h0own_sb[:], start=False, stop=True)
            psGB = pp.tile([128, 2 * HC], f32, tag="psG", bufs=2)
            for c in range(HC):
                nc.tensor.matmul(
                    psGB[:, c : c + 1],
                    wih_sb[:, (2 * HC + c) * 128 : (2 * HC + c + 1) * 128],
                    xsh_sb[:], start=True, stop=True)
                nc.tensor.matmul(
                    psGB[:, HC + c : HC + c + 1],
                    whh_sb[:, (2 * HC + c) * 128 : (2 * HC + c + 1) * 128],
                    h0own_sb[:], start=True, stop=True)
            gpart_sb = spool.tile([128, 4 * HC], f32)
            nc.vector.tensor_copy(gpart_sb[:, 0 : 2 * HC], psGA[:])
            nc.vector.tensor_copy(gpart_sb[:, 2 * HC : 4 * HC], psGB[:])

            ar_in = dram.tile([128, 4 * HC], f32)
            ar_out = dram.tile([128, 4 * HC], f32, addr_space="Shared")
            nc.gpsimd.dma_start(ar_in[:], gpart_sb[:])
            nc.gpsimd.collective_compute(
                "AllReduce", mybir.AluOpType.add, replica_groups=RG,
                ins=[ar_in[:].opt()], outs=[ar_out[:].opt()],
            )
            gfull_sb = spool.tile([128, 4 * HC], f32)
            nc.gpsimd.dma_start(gfull_sb[:], ar_out[:])

            # elementwise GRU on full [128, HC] chunk-layout tensors
            brz_sb = spool.tile([128, 2 * HC], f32)
            nc.vector.tensor_add(brz_sb[:], bih_sb[:, 0 : 2 * HC], bhh_sb[:, 0 : 2 * HC])
            rzin_sb = spool.tile([128, 2 * HC], f32)
            nc.vector.tensor_add(rzin_sb[:], gfull_sb[:, 0 : 2 * HC], brz_sb[:])
            rz_sb = spool.tile([128, 2 * HC], f32)
            nc.scalar.activation(
                rz_sb[:], rzin_sb[:], mybir.ActivationFunctionType.Sigmoid)
            hnb_sb = spool.tile([128, HC], f32)
            nc.vector.tensor_add(
                hnb_sb[:], gfull_sb[:, 3 * HC : 4 * HC], bhh_sb[:, 2 * HC : 3 * HC])
            rhn_sb = spool.tile([128, HC], f32)
            nc.vector.tensor_mul(rhn_sb[:], rz_sb[:, 0:HC], hnb_sb[:])
            t1_sb = spool.tile([128, HC], f32)
            nc.vector.tensor_add(t1_sb[:], gfull_sb[:, 2 * HC : 3 * HC], rhn_sb[:])
            t2_sb = spool.tile([128, HC], f32)
            nc.vector.tensor_add(t2_sb[:], t1_sb[:], bih_sb[:, 2 * HC : 3 * HC])
            n_sb = spool.tile([128, HC], f32)
            nc.scalar.activation(
                n_sb[:], t2_sb[:], mybir.ActivationFunctionType.Tanh)
            d_sb = spool.tile([128, HC], f32)
            nc.vector.tensor_sub(d_sb[:], h0c_sb[:], n_sb[:])
            zd_sb = spool.tile([128, HC], f32)
            nc.vector.tensor_mul(zd_sb[:], rz_sb[:, HC : 2 * HC], d_sb[:])
            hn_sb = spool.tile([128, HC], f32)
            nc.vector.tensor_add(hn_sb[:], n_sb[:], zd_sb[:])
            nc.scalar.dma_start(h_out[:], hn_sb[:])
            h_mm = cpool.tile([128, HC], wdt)
            nc.vector.tensor_copy(h_mm[:], hn_sb[:])

            # ---- out projection (streamed, vocab shard VS=6400, 16 tiles) ----
            logits0_sb = cpool.tile([16, TN], f32)
            for wc in range(NT // WG):
                w_tile = wpool.tile([128, WG * HC * TN], wdt, tag="wtile")
                nc.sync.dma_start(w_tile[:], wout_in[wc])
                for ti in range(WG):
                    t = wc * WG + ti
                    psT = pp.tile([1, TN], f32, tag="psT", bufs=2)
                    for k in range(HC):
                        nc.tensor.matmul(
                            psT[:],
                            h_mm[:, k : k + 1],
                            w_tile[:, (ti * HC + k) * TN : (ti * HC + k + 1) * TN],
                            start=(k == 0),
                            stop=(k == HC - 1),
                        )
                    # compute engines can't address partition t directly (32-part
                    # alignment) — stage on partition 0, DMA-scatter to row t
                    lrow = spool.tile([1, TN], f32, tag="lrow", bufs=3)
                    nc.vector.tensor_copy(lrow[:], psT[:])
                    nc.scalar.dma_start(logits0_sb[t : t + 1, :], lrow[:])
            # bias add (also applies the -1e4 padding bias)
            logits_sb = cpool.tile([16, TN], f32)
            nc.vector.tensor_add(logits_sb[:], logits0_sb[:], bout_sb[:])

            # ---- local log-softmax stats ----
            mx16 = spool.tile([16, 1], f32)
            nc.vector.reduce_max(mx16[:], logits_sb[:], axis=mybir.AxisListType.X)
            psS2 = pp.tile([128, 16], f32, tag="psS", bufs=2)
            nc.tensor.transpose(psS2[0:1, 0:16], mx16[:], ident[0:16, 0:16])
            mt_sb = spool.tile([1, 16], f32)
            nc.vector.tensor_copy(mt_sb[:], psS2[0:1, 0:16])
            mc = spool.tile([1, 1], f32)
            nc.vector.reduce_max(mc[:], mt_sb[:], axis=mybir.AxisListType.X)
            nmc = spool.tile([1, 1], f32)
            nc.vector.tensor_scalar_mul(nmc[:], mc[:], -1.0)
            psB = pp.tile([16, 1], f32, tag="psS", bufs=2)
            nc.tensor.matmul(psB[:], ones_row[0:1, 0:16], nmc[:], start=True, stop=True)
            nm16_sb = spool.tile([16, 1], f32)
            nc.vector.tensor_copy(nm16_sb[:], psB[:])
            e16 = spool.tile([16, TN], f32)
            zrow = spool.tile([16, 1], f32)
            nc.scalar.activation(
                e16[:], logits_sb[:], mybir.ActivationFunctionType.Exp,
                bias=nm16_sb[:, 0:1], accum_out=zrow[:],
            )
            psZc = pp.tile([1, 1], f32, tag="psS", bufs=2)
            nc.tensor.matmul(psZc[:], zrow[:], ones_col[0:16, 0:1], start=True, stop=True)
            stats_sb = spool.tile([1, 2], f32)
            nc.vector.tensor_copy(stats_sb[0:1, 0:1], mc[:])
            nc.vector.tensor_copy(stats_sb[0:1, 1:2], psZc[:])

            # ---- AllGather stats ----
            st_in = dram.tile([1, 2], f32)
            st_out = dram.tile([NCORES, 2], f32, addr_space="Shared")
            nc.gpsimd.dma_start(st_in[:], stats_sb[:])
            nc.gpsimd.collective_compute(
                "AllGather", mybir.AluOpType.bypass, replica_groups=RG,
                ins=[st_in[:].opt()], outs=[st_out[:].opt()],
            )
            s8_sb = spool.tile([NCORES, 2], f32)
            nc.gpsimd.dma_start(s8_sb[:], st_out[:])

            # global max M, then C = M + ln(sum_c Z_c exp(m_c - M))
            psM = pp.tile([128, 16], f32, tag="psS", bufs=2)
            nc.tensor.transpose(psM[0:1, 0:NCORES], s8_sb[:, 0:1], ident[0:NCORES, 0:NCORES])
            m1_sb = spool.tile([1, NCORES], f32)
            nc.vector.tensor_copy(m1_sb[:], psM[0:1, 0:NCORES])
            gM = spool.tile([1, 1], f32)
            nc.vector.reduce_max(gM[:], m1_sb[:], axis=mybir.AxisListType.X)
            ngM = spool.tile([1, 1], f32)
            nc.vector.tensor_scalar_mul(ngM[:], gM[:], -1.0)
            psB2 = pp.tile([NCORES, 1], f32, tag="psS", bufs=2)
            nc.tensor.matmul(psB2[:], ones_row[0:1, 0:NCORES], ngM[:], start=True, stop=True)
            ngM8_sb = spool.tile([NCORES, 1], f32)
            nc.vector.tensor_copy(ngM8_sb[:], psB2[:])
            e8 = spool.tile([NCORES, 1], f32)
            nc.scalar.activation(
                e8[:], s8_sb[:, 0:1], mybir.ActivationFunctionType.Exp,
                bias=ngM8_sb[:, 0:1])
            s8p = spool.tile([NCORES, 1], f32)
            nc.vector.tensor_mul(s8p[:], e8[:], s8_sb[:, 1:2])
            psZg = pp.tile([1, 1], f32, tag="psS", bufs=2)
            nc.tensor.matmul(psZg[:], s8p[:], ones_col[0:NCORES, 0:1], start=True, stop=True)
            lnZ = spool.tile([1, 1], f32)
            nc.scalar.activation(lnZ[:], psZg[:], mybir.ActivationFunctionType.Ln)
            C = spool.tile([1, 1], f32)
            nc.scalar.activation(
                C[:], lnZ[:], mybir.ActivationFunctionType.Identity,
                bias=gM[0:1, 0:1])
            nC = spool.tile([1, 1], f32)
            nc.vector.tensor_scalar_mul(nC[:], C[:], -1.0)
            psB3 = pp.tile([16, 1], f32, tag="psS", bufs=2)
            nc.tensor.matmul(psB3[:], ones_row[0:1, 0:16], nC[:], start=True, stop=True)
            nC16_sb = spool.tile([16, 1], f32)
            nc.vector.tensor_copy(nC16_sb[:], psB3[:])
            logp_sb = spool.tile([16, TN], f32)
            nc.scalar.activation(
                logp_sb[:], logits_sb[:], mybir.ActivationFunctionType.Identity,
                bias=nC16_sb[:, 0:1])
            nc.sync.dma_start(logp_out[:], logp_sb[:])

    nc.compile()
    return nc


def _prep_inputs(inp, hidden, encoder_outputs, emb_W, attn_W, attn_b,
                 comb_W, comb_b, W_ih, W_hh, b_ih, b_hh, out_W, out_b):
    """Shard/layout the full inputs into 8 per-core input maps."""
    f = np.float32
    idx = int(np.asarray(inp).ravel()[0])
    emb_row = np.asarray(emb_W[idx], dtype=f)                 # [1024]
    h0 = np.asarray(hidden, dtype=f).ravel()                  # [1024]
    enc = np.ascontiguousarray(np.asarray(encoder_outputs, dtype=f))  # [24,1024]

    def chunked_vec(v):
        # [1024] -> [128, 8] with [p, c] = v[c*128+p]
        return np.ascontiguousarray(v.reshape(HC, 128).T)

    emb_c = chunked_vec(emb_row)
    h0_c = chunked_vec(h0)

    # attn_W [24, 2048] -> [128, 16*24]
    aT = np.asarray(attn_W, dtype=f).T.reshape(16, 128, MAX_LEN)
    attnw = np.ascontiguousarray(aT.transpose(1, 0, 2).reshape(128, 16 * MAX_LEN))
    attnb = np.ascontiguousarray(np.asarray(attn_b, dtype=f).reshape(1, MAX_LEN))

    comb_W = np.asarray(comb_W, dtype=f)
    comb_b_a = np.asarray(comb_b, dtype=f)
    W_ih_a = np.asarray(W_ih, dtype=f)
    W_hh_a = np.asarray(W_hh, dtype=f)
    b_ih_a = np.asarray(b_ih, dtype=f)
    b_hh_a = np.asarray(b_hh, dtype=f)
    out_W_a = np.asarray(out_W, dtype=f)
    out_b_a = np.asarray(out_b, dtype=f)

    if WOUT_BF16:
        import ml_dtypes

        wout_dt = ml_dtypes.bfloat16
    else:
        wout_dt = f

    in_maps = []
    for j in range(NCORES):
        sl = slice(j * 128, (j + 1) * 128)
        # comb shard [128, 2048] -> [128(p), 16*128]
        cw = comb_W[sl]                                    # [128, 2048]
        cwT = cw.T.reshape(16, 128, 128)                   # [c, p, m]
        combw = np.ascontiguousarray(cwT.transpose(1, 0, 2).reshape(128, 16 * 128))
        combb = np.ascontiguousarray(comb_b_a[sl].reshape(128, 1))

        def gate_pack(W):
            # contraction shard: lhsT tile (g,c)[p, m] = W[g*1024+c*128+m, j*128+p]
            cols = []
            for g in range(3):
                A = W[g * NHID : (g + 1) * NHID, j * 128 : (j + 1) * 128]  # [1024, 128]
                B = A.reshape(HC, 128, 128)                                 # [c, m, p]
                cols.append(B.transpose(2, 0, 1).reshape(128, HC * 128))
            return np.ascontiguousarray(np.concatenate(cols, axis=1))

        wih = gate_pack(W_ih_a)
        whh = gate_pack(W_hh_a)

        def bias_pack(b):
            # [128, 3*HC]: col g*HC+c holds b[g*1024 + c*128 + p]
            return np.ascontiguousarray(
                b.reshape(3, HC, 128).transpose(2, 0, 1).reshape(128, 3 * HC))

        bih = bias_pack(b_ih_a)
        bhh = bias_pack(b_hh_a)

        # out_W vocab shard [VS, 1024] (zero-padded), bias shard with PAD_BIAS
        lo, hi = j * VS, min((j + 1) * VS, NOUT)
        nreal = max(0, hi - lo)
        wsh = np.zeros((VS, NHID), dtype=f)
        bsh = np.full((VS,), PAD_BIAS, dtype=f)
        if nreal > 0:
            wsh[:nreal] = out_W_a[lo:hi]
            bsh[:nreal] = out_b_a[lo:hi]
        WT = wsh.T                                         # [1024, 6400]
        # [NT, 128, HC*TN]: [t, p, k*TN+n] = WT[k*128+p, t*TN+n], grouped by WG
        warr = (
            WT.reshape(HC, 128, NT, TN).transpose(2, 1, 0, 3)
            .reshape(NT // WG, WG, 128, HC * TN).transpose(0, 2, 1, 3)
            .reshape(NT // WG, 128, WG * HC * TN)
        )
        warr = np.ascontiguousarray(warr.astype(wout_dt))
        barr = np.ascontiguousarray(bsh.reshape(16, TN))

        in_maps.append({
            "emb_in": emb_c, "h0c_in": h0_c,
            "h0own_in": np.ascontiguousarray(h0[sl].reshape(128, 1)),
            "enc_in": enc, "attnw_in": attnw, "attnb_in": attnb,
            "combw_in": combw, "combb_in": combb,
            "wih_in": wih, "whh_in": whh, "bih_in": bih, "bhh_in": bhh,
            "wout_in": warr, "bout_in": barr,
        })
    return in_maps


def run(trace=False, **inputs):
    from concourse.bass_utils import run_bass_kernel_spmd

    if "nc" not in _CACHE:
        _CACHE["nc"] = _build()
    nc = _CACHE["nc"]

    inputs.pop("encoder_output", None)  # unused by the reference computation
    in_maps = _prep_inputs(**inputs)
    res = run_bass_kernel_spmd(
        nc, in_maps, core_ids=list(range(NCORES)), trace=trace
    )

    logp = np.concatenate(
        [res.results[j]["logp_out"].reshape(-1) for j in range(NCORES)]
    )[:NOUT].reshape(1, NOUT).astype(np.float32)
    # h_out is [128, HC] chunk layout, full h' replicated on every core
    h = res.results[0]["h_out"].T.reshape(1, 1, NHID).astype(np.float32)
    attn = res.results[0]["attn_out"].reshape(1, MAX_LEN).astype(np.float32)
    return (logp, h, attn), res


def kernel(**inputs):
    out, _ = run(trace=bool(os.environ.get("KERNEL_TRACE")), **inputs)
    return out


# revision 21
# speedup vs baseline: 2.0624x; 1.2634x over previous
"""AttnDecoderRNN step on 8 TRN2 NeuronCores (Bass/Tile).

Sharding (per sharding hint): vocab-parallel out projection (embedding row
handled as a host-side gather/shard selection), hidden-sharded comb/GRU
matmuls, replicated attention. Collectives: AllGather of x (post-comb relu),
AllGather of h', AllGather of per-core log-softmax stats (max, sumexp).

Engine assignment (avoids the gpsimd/SWDGE backlog that delayed collective
triggers): gpsimd carries ONLY collective bounces + triggers + gathered
loads; small inputs ride the scalar HWDGE ring; big weights go first on the
sync ring ahead of the streamed out_W tiles.

Shapes: NHID=1024, NOUT=50257, MAX_LEN=24, batch=1.
Per-core vocab shard: VS=6400 (8*6400=51200 >= 50257; padding gets bias -1e4).
"""
import os
import sys
import types
import contextlib
import ctypes

import numpy as np

# ---------------------------------------------------------------------------
# antenv.axon_hooks shim: the container's antenv stub lacks this module, but
# concourse.bass_utils imports it when tracing is requested (BASS_TRACE=1).
# Provide it, with the ctypes NTFF profile hook libaxon exposes.
# ---------------------------------------------------------------------------
_HOOK = [None]


def _install_axon_hook_shim():
    if "antenv.axon_hooks" not in sys.modules:
        mod = types.ModuleType("antenv.axon_hooks")

        def set_axon_ntff_profile_hook(h):
            _HOOK[0] = h

        def get_axon_ntff_profile_hook():
            return _HOOK[0]

        mod.set_axon_ntff_profile_hook = set_axon_ntff_profile_hook
        mod.get_axon_ntff_profile_hook = get_axon_ntff_profile_hook
        sys.modules["antenv.axon_hooks"] = mod
        try:
            import antenv

            antenv.axon_hooks = mod
        except ImportError:
            pass
    if _HOOK[0] is None:
        so_path = "/opt/axon/libaxon_pjrt.so"
        try:
            lib = ctypes.CDLL(so_path)
        except OSError:
            return
        if not hasattr(lib, "axon_start_nrt_profile"):
            return
        lib.axon_start_nrt_profile.argtypes = [
            ctypes.POINTER(ctypes.c_int64),
            ctypes.c_size_t,
        ]
        lib.axon_start_nrt_profile.restype = ctypes.c_int64
        lib.axon_stop_nrt_profile.argtypes = [ctypes.c_char_p]
        lib.axon_stop_nrt_profile.restype = ctypes.c_int64

        @contextlib.contextmanager
        def _hook(output_dir, device_ids):
            import jax

            jax.devices()
            if device_ids:
                ids = (ctypes.c_int64 * len(device_ids))(*device_ids)
                rc = lib.axon_start_nrt_profile(ids, len(device_ids))
            else:
                rc = lib.axon_start_nrt_profile(None, 0)
            if rc != 0:
                raise RuntimeError(f"axon_start_nrt_profile rc={rc}")
            try:
                yield
            finally:
                n = lib.axon_stop_nrt_profile(str(output_dir).encode())
                print(f"profile: {n} file(s) -> {output_dir}", file=sys.stderr)

        sys.modules["antenv.axon_hooks"].set_axon_ntff_profile_hook(_hook)


_install_axon_hook_shim()

NCORES = 8
NHID = 1024
NOUT = 50257
MAX_LEN = 24
HC = NHID // 128          # 8 hidden chunks of 128
VS = 6400                 # vocab rows per core (padded)
TN = 400                  # out-projection free-dim tile (PSUM bank limit: 512 f32)
NT = VS // TN             # 16 logical tiles per core
WG = 4                    # logical tiles per W DMA chunk
PAD_BIAS = -1.0e4         # bias on padded vocab rows: exp() underflows to 0
WOUT_BF16 = os.environ.get("WOUT_DTYPE", "bf16") == "bf16"

_CACHE = {}


def _build():
    import concourse.bass as bass
    import concourse.tile as tile
    from concourse import bacc, mybir, masks
    from contextlib import ExitStack

    f32 = mybir.dt.float32
    wdt = mybir.dt.bfloat16 if WOUT_BF16 else f32

    nc = bacc.Bacc(
        "TRN2",
        target_bir_lowering=False,
        debug=False,
        enable_asserts=True,
        num_devices=NCORES,
    )

    # ---- I/O ----
    emb_in = nc.dram_tensor("emb_in", [128, HC], f32, kind="ExternalInput")
    h0c_in = nc.dram_tensor("h0c_in", [128, HC], f32, kind="ExternalInput")
    h0own_in = nc.dram_tensor("h0own_in", [128, 1], f32, kind="ExternalInput")
    enc_in = nc.dram_tensor("enc_in", [MAX_LEN, NHID], f32, kind="ExternalInput")
    attnw_in = nc.dram_tensor("attnw_in", [128, 16 * MAX_LEN], f32, kind="ExternalInput")
    attnb_in = nc.dram_tensor("attnb_in", [1, MAX_LEN], f32, kind="ExternalInput")
    combw_in = nc.dram_tensor("combw_in", [128, 16 * 128], f32, kind="ExternalInput")
    combb_in = nc.dram_tensor("combb_in", [1, 128], f32, kind="ExternalInput")
    wih_in = nc.dram_tensor("wih_in", [128, 3 * NHID], f32, kind="ExternalInput")
    whh_in = nc.dram_tensor("whh_in", [128, 3 * NHID], f32, kind="ExternalInput")
    bih_in = nc.dram_tensor("bih_in", [128, 3 * HC], f32, kind="ExternalInput")
    bhh_in = nc.dram_tensor("bhh_in", [128, 3 * HC], f32, kind="ExternalInput")
    wout_in = nc.dram_tensor(
        "wout_in", [NT // WG, 128, WG * HC * TN], wdt, kind="ExternalInput"
    )
    bout_in = nc.dram_tensor("bout_in", [1, VS], f32, kind="ExternalInput")

    logp_out = nc.dram_tensor("logp_out", [16, TN], f32, kind="ExternalOutput")
    h_out = nc.dram_tensor("h_out", [128, HC], f32, kind="ExternalOutput")
    attn_out = nc.dram_tensor("attn_out", [1, MAX_LEN], f32, kind="ExternalOutput")

    RG = [list(range(NCORES))]

    with tile.TileContext(nc) as tc:
        with ExitStack() as ctx:
            wpool = ctx.enter_context(tc.tile_pool(name="wpool", bufs=NT // WG))
            cpool = ctx.enter_context(tc.tile_pool(name="cpool", bufs=1))
            spool = ctx.enter_context(tc.tile_pool(name="spool", bufs=1))
            pp = ctx.enter_context(tc.tile_pool(name="pp", bufs=2, space="PSUM"))
            dram = ctx.enter_context(tc.tile_pool(name="dram", bufs=1, space="DRAM"))

            # ---- constants / inputs to SBUF ----
            # chain-critical inputs FIRST on the sync ring (ahead of the W
            # stream); non-critical small ones on the scalar ring
            ident = cpool.tile([128, 128], f32)
            masks.make_identity(nc, ident[:])
            ones_row = cpool.tile([1, 128], f32)   # [1,P] lhsT for broadcasts
            nc.gpsimd.memset(ones_row[:], 1.0)

            emb_sb = cpool.tile([128, HC], f32)
            nc.sync.dma_start(emb_sb[:], emb_in[:])
            h0c_sb = cpool.tile([128, HC], f32)
            nc.sync.dma_start(h0c_sb[:], h0c_in[:])
            h0own_sb = cpool.tile([128, 1], f32)
            nc.sync.dma_start(h0own_sb[:], h0own_in[:])
            enc_sb = cpool.tile([MAX_LEN, NHID], f32)
            nc.sync.dma_start(enc_sb[:], enc_in[:])
            attnw_sb = cpool.tile([128, 16 * MAX_LEN], f32)
            nc.sync.dma_start(attnw_sb[:], attnw_in[:])
            attnb_sb = cpool.tile([1, MAX_LEN], f32)
            nc.sync.dma_start(attnb_sb[:], attnb_in[:])
            combb_sb = cpool.tile([1, 128], f32)
            nc.sync.dma_start(combb_sb[:], combb_in[:])
            combw_sb = cpool.tile([128, 16 * 128], f32)
            nc.sync.dma_start(combw_sb[:], combw_in[:])
            wih_sb = cpool.tile([128, 3 * NHID], f32)
            nc.sync.dma_start(wih_sb[:], wih_in[:])
            whh_sb = cpool.tile([128, 3 * NHID], f32)
            nc.sync.dma_start(whh_sb[:], whh_in[:])

            bih_sb = cpool.tile([128, 3 * HC], f32)
            nc.scalar.dma_start(bih_sb[:], bih_in[:])
            bhh_sb = cpool.tile([128, 3 * HC], f32)
            nc.scalar.dma_start(bhh_sb[:], bhh_in[:])


            # ---- attention (replicated) ----
            psA = pp.tile([128, MAX_LEN], f32, tag="psA", bufs=2)
            alog_ps = psA[0:1, 0:MAX_LEN]
            for c in range(HC):
                nc.tensor.matmul(
                    alog_ps,
                    emb_sb[:, c : c + 1],
                    attnw_sb[:, c * MAX_LEN : (c + 1) * MAX_LEN],
                    start=(c == 0),
                    stop=False,
                )
            for c in range(HC):
                nc.tensor.matmul(
                    alog_ps,
                    h0c_sb[:, c : c + 1],
                    attnw_sb[:, (HC + c) * MAX_LEN : (HC + c + 1) * MAX_LEN],
                    start=False,
                    stop=(c == HC - 1),
                )
            alog_sb = spool.tile([1, MAX_LEN], f32)
            nc.vector.tensor_add(alog_sb[:], alog_ps, attnb_sb[:])
            amax = spool.tile([1, 1], f32)
            nc.vector.reduce_max(amax[:], alog_sb[:], axis=mybir.AxisListType.X)
            namax = spool.tile([1, 1], f32)
            nc.vector.tensor_scalar_mul(namax[:], amax[:], -1.0)
            probs = spool.tile([1, MAX_LEN], f32)
            sume = spool.tile([1, 1], f32)
            nc.scalar.activation(
                probs[:], alog_sb[:], mybir.ActivationFunctionType.Exp,
                bias=namax[0:1, 0:1], accum_out=sume[:],
            )
            rinv = spool.tile([1, 1], f32)
            nc.vector.reciprocal(rinv[:], sume[:])
            attnp_sb = spool.tile([1, MAX_LEN], f32)
            nc.scalar.mul(attnp_sb[:], probs[:], rinv[0:1, 0:1])
            nc.scalar.dma_start(attn_out[:], attnp_sb[:])

            # transpose attn probs -> [24, 1]
            psS = pp.tile([128, 16], f32, tag="psS", bufs=2)
            nc.tensor.transpose(psS[0:MAX_LEN, 0:1], attnp_sb[:], ident[0:1, 0:1])
            awt_sb = spool.tile([MAX_LEN, 1], f32)
            nc.vector.tensor_copy(awt_sb[:], psS[0:MAX_LEN, 0:1])

            # attn_applied chunks: [128, HC]
            psA2 = pp.tile([128, MAX_LEN], f32, tag="psA", bufs=2)
            for c in range(HC):
                nc.tensor.matmul(
                    psA2[:, c : c + 1],
                    enc_sb[0:MAX_LEN, c * 128 : (c + 1) * 128],
                    awt_sb[:],
                    start=True,
                    stop=True,
                )
            aap_sb = spool.tile([128, HC], f32)
            nc.vector.tensor_copy(aap_sb[:], psA2[:, 0:HC])

            # ---- comb (sharded, thin-stationary): x row = relu(cat @ W_sh.T + b)
            # lhsT = cat chunk [128,1] (trivial weight load), rhs = W tile.
            psC = pp.tile([1, 512], f32, tag="psG", bufs=2)
            psC_ap = psC[0:1, 0:128]
            for c in range(HC):
                nc.tensor.matmul(
                    psC_ap, emb_sb[:, c : c + 1],
                    combw_sb[:, c * 128 : (c + 1) * 128],
                    start=(c == 0), stop=False)
            for c in range(HC):
                nc.tensor.matmul(
                    psC_ap, aap_sb[:, c : c + 1],
                    combw_sb[:, (HC + c) * 128 : (HC + c + 1) * 128],
                    start=False, stop=False)
            # bias via K=1 ones matmul, then relu
            nc.tensor.matmul(psC_ap, ones_row[0:1, 0:1], combb_sb[:],
                             start=False, stop=True)
            xrow_sb = spool.tile([1, 128], f32)
            nc.scalar.activation(
                xrow_sb[:], psC_ap, mybir.ActivationFunctionType.Relu)
            # transpose x row -> [128, 1] for use as stationary operand
            psX = pp.tile([128, 48], f32, tag="psA", bufs=2)
            nc.tensor.transpose(psX[:, 0:1], xrow_sb[:], ident[0:1, 0:1])
            xsh_sb = spool.tile([128, 1], f32)
            nc.vector.tensor_copy(xsh_sb[:], psX[:, 0:1])

            # ---- GRU, contraction-sharded with weights as the moving
            # operand: partial gi = x_chunk.T @ W_ih[:, j].T -> [1, 3072],
            # same for gh; AllReduce(add) of [1, 6144]; transpose-load to
            # [128, 48] chunk layout; elementwise GRU gives FULL h'
            # replicated on every core (no h all-gather).
            gpart_sb = spool.tile([1, 2 * 3 * NHID], f32)
            for n in range(6):
                psGI = pp.tile([1, 512], f32, tag="psG", bufs=2)
                nc.tensor.matmul(
                    psGI[:], xsh_sb[:], wih_sb[:, n * 512 : (n + 1) * 512],
                    start=True, stop=True)
                nc.vector.tensor_copy(gpart_sb[0:1, n * 512 : (n + 1) * 512], psGI[:])
            for n in range(6):
                psGH = pp.tile([1, 512], f32, tag="psG", bufs=2)
                nc.tensor.matmul(
                    psGH[:], h0own_sb[:], whh_sb[:, n * 512 : (n + 1) * 512],
                    start=True, stop=True)
                nc.vector.tensor_copy(
                    gpart_sb[0:1, 3 * NHID + n * 512 : 3 * NHID + (n + 1) * 512],
                    psGH[:])

            ar_in = dram.tile([1, 2 * 3 * NHID], f32)
            ar_out = dram.tile([1, 2 * 3 * NHID], f32, addr_space="Shared")
            nc.gpsimd.dma_start(ar_in[:], gpart_sb[:])
            nc.gpsimd.collective_compute(
                "AllReduce", mybir.AluOpType.add, replica_groups=RG,
                ins=[ar_in[:].opt()], outs=[ar_out[:].opt()],
            )
            # load as [48, 128] (contiguous rows) and transpose to [128, 48]
            g48_sb = spool.tile([48, 128], f32)
            nc.gpsimd.dma_start(g48_sb[:], ar_out[:].rearrange("o (a b) -> (o a) b", b=128))
            psG48 = pp.tile([128, 48], f32, tag="psA", bufs=2)
            nc.tensor.transpose(psG48[:], g48_sb[:], ident[0:48, 0:48])
            gf_sb = spool.tile([128, 48], f32)
            nc.vector.tensor_copy(gf_sb[:], psG48[:])
            # cols: 0:8 gi_r, 8:16 gi_z, 16:24 gi_n, 24:32 gh_r, 32:40 gh_z, 40:48 gh_n

            # elementwise GRU on [128, HC] chunk-layout tensors
            brz_sb = spool.tile([128, 2 * HC], f32)
            nc.vector.tensor_add(brz_sb[:], bih_sb[:, 0 : 2 * HC], bhh_sb[:, 0 : 2 * HC])
            rz0_sb = spool.tile([128, 2 * HC], f32)
            nc.vector.tensor_add(rz0_sb[:], gf_sb[:, 0 : 2 * HC], gf_sb[:, 3 * HC : 5 * HC])
            rzin_sb = spool.tile([128, 2 * HC], f32)
            nc.vector.tensor_add(rzin_sb[:], rz0_sb[:], brz_sb[:])
            rz_sb = spool.tile([128, 2 * HC], f32)
            nc.scalar.activation(
                rz_sb[:], rzin_sb[:], mybir.ActivationFunctionType.Sigmoid)
            hnb_sb = spool.tile([128, HC], f32)
            nc.vector.tensor_add(
                hnb_sb[:], gf_sb[:, 5 * HC : 6 * HC], bhh_sb[:, 2 * HC : 3 * HC])
            rhn_sb = spool.tile([128, HC], f32)
            nc.vector.tensor_mul(rhn_sb[:], rz_sb[:, 0:HC], hnb_sb[:])
            t1_sb = spool.tile([128, HC], f32)
            nc.vector.tensor_add(t1_sb[:], gf_sb[:, 2 * HC : 3 * HC], rhn_sb[:])
            t2_sb = spool.tile([128, HC], f32)
            nc.vector.tensor_add(t2_sb[:], t1_sb[:], bih_sb[:, 2 * HC : 3 * HC])
            n_sb = spool.tile([128, HC], f32)
            nc.scalar.activation(
                n_sb[:], t2_sb[:], mybir.ActivationFunctionType.Tanh)
            d_sb = spool.tile([128, HC], f32)
            nc.vector.tensor_sub(d_sb[:], h0c_sb[:], n_sb[:])
            zd_sb = spool.tile([128, HC], f32)
            nc.vector.tensor_mul(zd_sb[:], rz_sb[:, HC : 2 * HC], d_sb[:])
            hn_sb = spool.tile([128, HC], f32)
            nc.vector.tensor_add(hn_sb[:], n_sb[:], zd_sb[:])
            nc.scalar.dma_start(h_out[:], hn_sb[:])
            h_mm = cpool.tile([128, HC], wdt)
            nc.vector.tensor_copy(h_mm[:], hn_sb[:])

            # ---- out projection (streamed, vocab shard VS=6400, 16 tiles)
            # bias folded into the matmul accumulation via a K=1 ones matmul;
            # per-tile online softmax stats on partition 0 overlap the stream.
            logits_sb = cpool.tile([16, TN], f32)
            mrow = spool.tile([1, NT], f32)
            nmrow = spool.tile([1, NT], f32)
            zrow = spool.tile([1, NT], f32)
            for wc in range(NT // WG):
                w_tile = wpool.tile([128, WG * HC * TN], wdt, tag="wtile")
                nc.sync.dma_start(w_tile[:], wout_in[wc])
                for ti in range(WG):
                    t = wc * WG + ti
                    psT = pp.tile([1, TN], f32, tag="psT", bufs=2)
                    for k in range(HC):
                        nc.tensor.matmul(
                            psT[:],
                            h_mm[:, k : k + 1],
                            w_tile[:, (ti * HC + k) * TN : (ti * HC + k + 1) * TN],
                            start=(k == 0),
                            stop=False,
                        )
                    bout_t = spool.tile([1, TN], f32, tag="bout_t", bufs=2)
                    nc.scalar.dma_start(bout_t[:], bout_in[0:1, t * TN : (t + 1) * TN])
                    nc.tensor.matmul(
                        psT[:], ones_row[0:1, 0:1], bout_t[:],
                        start=False, stop=True)
                    # compute engines can't address partition t directly (32-part
                    # alignment) — stage on partition 0, DMA-scatter to row t
                    lrow = spool.tile([1, TN], f32, tag="lrow", bufs=3)
                    nc.vector.tensor_copy(lrow[:], psT[:])
                    nc.scalar.dma_start(logits_sb[t : t + 1, :], lrow[:])
                    # online per-tile stats
                    nc.vector.reduce_max(
                        mrow[0:1, t : t + 1], lrow[:], axis=mybir.AxisListType.X)
                    nc.vector.tensor_scalar_mul(
                        nmrow[0:1, t : t + 1], mrow[0:1, t : t + 1], -1.0)
                    e_scr = spool.tile([1, TN], f32, tag="escr", bufs=2)
                    nc.scalar.activation(
                        e_scr[:], lrow[:], mybir.ActivationFunctionType.Exp,
                        bias=nmrow[0:1, t : t + 1],
                        accum_out=zrow[0:1, t : t + 1])

            # ---- combine the 16 per-tile stats (partition 0) ----
            mloc = spool.tile([1, 1], f32)
            nc.vector.reduce_max(mloc[:], mrow[:], axis=mybir.AxisListType.X)
            nmloc = spool.tile([1, 1], f32)
            nc.vector.tensor_scalar_mul(nmloc[:], mloc[:], -1.0)
            et16 = spool.tile([1, NT], f32)
            nc.scalar.activation(
                et16[:], mrow[:], mybir.ActivationFunctionType.Exp,
                bias=nmloc[0:1, 0:1])
            sz16 = spool.tile([1, NT], f32)
            nc.vector.tensor_mul(sz16[:], et16[:], zrow[:])
            stats_sb = spool.tile([1, 2], f32)
            nc.vector.tensor_copy(stats_sb[0:1, 0:1], mloc[:])
            nc.vector.reduce_sum(
                stats_sb[0:1, 1:2], sz16[:], axis=mybir.AxisListType.X)

            # ---- AllGather stats ----
            st_in = dram.tile([1, 2], f32)
            st_out = dram.tile([NCORES, 2], f32, addr_space="Shared")
            nc.gpsimd.dma_start(st_in[:], stats_sb[:])
            nc.gpsimd.collective_compute(
                "AllGather", mybir.AluOpType.bypass, replica_groups=RG,
                ins=[st_in[:].opt()], outs=[st_out[:].opt()],
            )
            s16_sb = spool.tile([1, 2 * NCORES], f32)
            nc.gpsimd.dma_start(
                s16_sb[:], st_out[:].rearrange("(o a) b -> o (a b)", o=1))

            # global max M, then C = M + ln(sum_c Z_c exp(m_c - M))
            s16v = s16_sb[:].rearrange("p (a b) -> p a b", b=2)
            gM = spool.tile([1, 1], f32)
            nc.vector.reduce_max(gM[:], s16v[:, :, 0:1], axis=mybir.AxisListType.XY)
            ngM = spool.tile([1, 1], f32)
            nc.vector.tensor_scalar_mul(ngM[:], gM[:], -1.0)
            e8 = spool.tile([1, NCORES], f32)
            nc.scalar.activation(
                e8[:], s16v[:, :, 0:1], mybir.ActivationFunctionType.Exp,
                bias=ngM[0:1, 0:1])
            s8p = spool.tile([1, NCORES], f32)
            nc.vector.tensor_mul(s8p[:], e8[:], s16v[:, :, 1:2])
            Zg = spool.tile([1, 1], f32)
            nc.vector.reduce_sum(Zg[:], s8p[:], axis=mybir.AxisListType.X)
            lnZ = spool.tile([1, 1], f32)
            nc.scalar.activation(lnZ[:], Zg[:], mybir.ActivationFunctionType.Ln)
            C = spool.tile([1, 1], f32)
            nc.scalar.activation(
                C[:], lnZ[:], mybir.ActivationFunctionType.Identity,
                bias=gM[0:1, 0:1])
            nC = spool.tile([1, 1], f32)
            nc.vector.tensor_scalar_mul(nC[:], C[:], -1.0)
            psB3 = pp.tile([16, 1], f32, tag="psS", bufs=2)
            nc.tensor.matmul(psB3[:], ones_row[0:1, 0:16], nC[:], start=True, stop=True)
            nC16_sb = spool.tile([16, 1], f32)
            nc.vector.tensor_copy(nC16_sb[:], psB3[:])
            logp_sb = spool.tile([16, TN], f32)
            nc.scalar.activation(
                logp_sb[:], logits_sb[:], mybir.ActivationFunctionType.Identity,
                bias=nC16_sb[:, 0:1])
            nc.sync.dma_start(logp_out[:], logp_sb[:])

    nc.compile()
    return nc


def _prep_inputs(inp, hidden, encoder_outputs, emb_W, attn_W, attn_b,
                 comb_W, comb_b, W_ih, W_hh, b_ih, b_hh, out_W, out_b):
    """Shard/layout the full inputs into 8 per-core input maps."""
    f = np.float32
    idx = int(np.asarray(inp).ravel()[0])
    emb_row = np.asarray(emb_W[idx], dtype=f)                 # [1024]
    h0 = np.asarray(hidden, dtype=f).ravel()                  # [1024]
    enc = np.ascontiguousarray(np.asarray(encoder_outputs, dtype=f))  # [24,1024]

    def chunked_vec(v):
        # [1024] -> [128, 8] with [p, c] = v[c*128+p]
        return np.ascontiguousarray(v.reshape(HC, 128).T)

    emb_c = chunked_vec(emb_row)
    h0_c = chunked_vec(h0)

    # attn_W [24, 2048] -> [128, 16*24]
    aT = np.asarray(attn_W, dtype=f).T.reshape(16, 128, MAX_LEN)
    attnw = np.ascontiguousarray(aT.transpose(1, 0, 2).reshape(128, 16 * MAX_LEN))
    attnb = np.ascontiguousarray(np.asarray(attn_b, dtype=f).reshape(1, MAX_LEN))

    comb_W = np.asarray(comb_W, dtype=f)
    comb_b_a = np.asarray(comb_b, dtype=f)
    W_ih_a = np.asarray(W_ih, dtype=f)
    W_hh_a = np.asarray(W_hh, dtype=f)
    b_ih_a = np.asarray(b_ih, dtype=f)
    b_hh_a = np.asarray(b_hh, dtype=f)
    out_W_a = np.asarray(out_W, dtype=f)
    out_b_a = np.asarray(out_b, dtype=f)

    if WOUT_BF16:
        import ml_dtypes

        wout_dt = ml_dtypes.bfloat16
    else:
        wout_dt = f

    in_maps = []
    for j in range(NCORES):
        sl = slice(j * 128, (j + 1) * 128)
        # comb shard [128, 2048] -> [128(p), 16*128]
        cw = comb_W[sl]                                    # [128, 2048]
        cwT = cw.T.reshape(16, 128, 128)                   # [c, p, m]
        combw = np.ascontiguousarray(cwT.transpose(1, 0, 2).reshape(128, 16 * 128))
        combb = np.ascontiguousarray(comb_b_a[sl].reshape(1, 128))

        def gate_pack(W):
            # moving-operand layout: rhs[p, q] = W[q, j*128+p]
            return np.ascontiguousarray(W[:, j * 128 : (j + 1) * 128].T)

        wih = gate_pack(W_ih_a)
        whh = gate_pack(W_hh_a)

        def bias_pack(b):
            # [128, 3*HC]: col g*HC+c holds b[g*1024 + c*128 + p]
            return np.ascontiguousarray(
                b.reshape(3, HC, 128).transpose(2, 0, 1).reshape(128, 3 * HC))

        bih = bias_pack(b_ih_a)
        bhh = bias_pack(b_hh_a)

        # out_W vocab shard [VS, 1024] (zero-padded), bias shard with PAD_BIAS
        lo, hi = j * VS, min((j + 1) * VS, NOUT)
        nreal = max(0, hi - lo)
        wsh = np.zeros((VS, NHID), dtype=f)
        bsh = np.full((VS,), PAD_BIAS, dtype=f)
        if nreal > 0:
            wsh[:nreal] = out_W_a[lo:hi]
            bsh[:nreal] = out_b_a[lo:hi]
        WT = wsh.T                                         # [1024, 6400]
        # [NT, 128, HC*TN]: [t, p, k*TN+n] = WT[k*128+p, t*TN+n], grouped by WG
        warr = (
            WT.reshape(HC, 128, NT, TN).transpose(2, 1, 0, 3)
            .reshape(NT // WG, WG, 128, HC * TN).transpose(0, 2, 1, 3)
            .reshape(NT // WG, 128, WG * HC * TN)
        )
        warr = np.ascontiguousarray(warr.astype(wout_dt))
        barr = np.ascontiguousarray(bsh.reshape(1, VS))

        in_maps.append({
            "emb_in": emb_c, "h0c_in": h0_c,
            "h0own_in": np.ascontiguousarray(h0[sl].reshape(128, 1)),
            "enc_in": enc, "attnw_in": attnw, "attnb_in": attnb,
            "combw_in": combw, "combb_in": combb,
            "wih_in": wih, "whh_in": whh, "bih_in": bih, "bhh_in": bhh,
            "wout_in": warr, "bout_in": barr,
        })
    return in_maps


def run(trace=False, **inputs):
    from concourse.bass_utils import run_bass_kernel_spmd

    if "nc" not in _CACHE:
        _CACHE["nc"] = _build()
    nc = _CACHE["nc"]

    inputs.pop("encoder_output", None)  # unused by the reference computation
    in_maps = _prep_inputs(**inputs)
    res = run_bass_kernel_spmd(
        nc, in_maps, core_ids=list(range(NCORES)), trace=trace
    )

    logp = np.concatenate(
        [res.results[j]["logp_out"].reshape(-1) for j in range(NCORES)]
    )[:NOUT].reshape(1, NOUT).astype(np.float32)
    # h_out is [128, HC] chunk layout, full h' replicated on every core
    h = res.results[0]["h_out"].T.reshape(1, 1, NHID).astype(np.float32)
    attn = res.results[0]["attn_out"].reshape(1, MAX_LEN).astype(np.float32)
    return (logp, h, attn), res


def kernel(**inputs):
    out, _ = run(trace=bool(os.environ.get("KERNEL_TRACE")), **inputs)
    return out


# revision 23
# speedup vs baseline: 2.1312x; 1.0334x over previous
"""AttnDecoderRNN step on 8 TRN2 NeuronCores (Bass/Tile).

Sharding (per sharding hint): vocab-parallel out projection (embedding row
handled as a host-side gather/shard selection), hidden-sharded comb/GRU
matmuls, replicated attention. Collectives: AllGather of x (post-comb relu),
AllGather of h', AllGather of per-core log-softmax stats (max, sumexp).

Engine assignment (avoids the gpsimd/SWDGE backlog that delayed collective
triggers): gpsimd carries ONLY collective bounces + triggers + gathered
loads; small inputs ride the scalar HWDGE ring; big weights go first on the
sync ring ahead of the streamed out_W tiles.

Shapes: NHID=1024, NOUT=50257, MAX_LEN=24, batch=1.
Per-core vocab shard: VS=6400 (8*6400=51200 >= 50257; padding gets bias -1e4).
"""
import os
import sys
import types
import contextlib
import ctypes

import numpy as np

# ---------------------------------------------------------------------------
# antenv.axon_hooks shim: the container's antenv stub lacks this module, but
# concourse.bass_utils imports it when tracing is requested (BASS_TRACE=1).
# Provide it, with the ctypes NTFF profile hook libaxon exposes.
# ---------------------------------------------------------------------------
_HOOK = [None]


def _install_axon_hook_shim():
    if "antenv.axon_hooks" not in sys.modules:
        mod = types.ModuleType("antenv.axon_hooks")

        def set_axon_ntff_profile_hook(h):
            _HOOK[0] = h

        def get_axon_ntff_profile_hook():
            return _HOOK[0]

        mod.set_axon_ntff_profile_hook = set_axon_ntff_profile_hook
        mod.get_axon_ntff_profile_hook = get_axon_ntff_profile_hook
        sys.modules["antenv.axon_hooks"] = mod
        try:
            import antenv

            antenv.axon_hooks = mod
        except ImportError:
            pass
    if _HOOK[0] is None:
        so_path = "/opt/axon/libaxon_pjrt.so"
        try:
            lib = ctypes.CDLL(so_path)
        except OSError:
            return
        if not hasattr(lib, "axon_start_nrt_profile"):
            return
        lib.axon_start_nrt_profile.argtypes = [
            ctypes.POINTER(ctypes.c_int64),
            ctypes.c_size_t,
        ]
        lib.axon_start_nrt_profile.restype = ctypes.c_int64
        lib.axon_stop_nrt_profile.argtypes = [ctypes.c_char_p]
        lib.axon_stop_nrt_profile.restype = ctypes.c_int64

        @contextlib.contextmanager
        def _hook(output_dir, device_ids):
            import jax

            jax.devices()
            if device_ids:
                ids = (ctypes.c_int64 * len(device_ids))(*device_ids)
                rc = lib.axon_start_nrt_profile(ids, len(device_ids))
            else:
                rc = lib.axon_start_nrt_profile(None, 0)
            if rc != 0:
                raise RuntimeError(f"axon_start_nrt_profile rc={rc}")
            try:
                yield
            finally:
                n = lib.axon_stop_nrt_profile(str(output_dir).encode())
                print(f"profile: {n} file(s) -> {output_dir}", file=sys.stderr)

        sys.modules["antenv.axon_hooks"].set_axon_ntff_profile_hook(_hook)


_install_axon_hook_shim()

NCORES = 8
NHID = 1024
NOUT = 50257
MAX_LEN = 24
HC = NHID // 128          # 8 hidden chunks of 128
VS = 6400                 # vocab rows per core (padded)
TN = 400                  # out-projection free-dim tile (PSUM bank limit: 512 f32)
NT = VS // TN             # 16 logical tiles per core
WG = 4                    # logical tiles per W DMA chunk
PAD_BIAS = -1.0e4         # bias on padded vocab rows: exp() underflows to 0
WOUT_BF16 = os.environ.get("WOUT_DTYPE", "bf16") == "bf16"

_CACHE = {}


def _build():
    import concourse.bass as bass
    import concourse.tile as tile
    from concourse import bacc, mybir, masks
    from contextlib import ExitStack

    f32 = mybir.dt.float32
    wdt = mybir.dt.bfloat16 if WOUT_BF16 else f32

    nc = bacc.Bacc(
        "TRN2",
        target_bir_lowering=False,
        debug=False,
        enable_asserts=True,
        num_devices=NCORES,
    )

    # ---- I/O ----
    pk_in = nc.dram_tensor("pk_in", [128, 65], f32, kind="ExternalInput")
    pr_in = nc.dram_tensor("pr_in", [1, 152], f32, kind="ExternalInput")
    enc_in = nc.dram_tensor("enc_in", [MAX_LEN, NHID], f32, kind="ExternalInput")
    attnw_in = nc.dram_tensor("attnw_in", [128, 16 * MAX_LEN], f32, kind="ExternalInput")
    combw_in = nc.dram_tensor("combw_in", [128, 16 * 128], f32, kind="ExternalInput")
    wih_in = nc.dram_tensor("wih_in", [128, 3 * NHID], f32, kind="ExternalInput")
    whh_in = nc.dram_tensor("whh_in", [128, 3 * NHID], f32, kind="ExternalInput")
    wout_in = nc.dram_tensor(
        "wout_in", [NT // WG, 128, WG * HC * TN], wdt, kind="ExternalInput"
    )
    bout_in = nc.dram_tensor("bout_in", [1, VS], f32, kind="ExternalInput")

    logp_out = nc.dram_tensor("logp_out", [16, TN], f32, kind="ExternalOutput")
    h_out = nc.dram_tensor("h_out", [128, HC], f32, kind="ExternalOutput")
    attn_out = nc.dram_tensor("attn_out", [1, MAX_LEN], f32, kind="ExternalOutput")

    RG = [list(range(NCORES))]

    with tile.TileContext(nc) as tc:
        with ExitStack() as ctx:
            wpool = ctx.enter_context(tc.tile_pool(name="wpool", bufs=3))
            cpool = ctx.enter_context(tc.tile_pool(name="cpool", bufs=1))
            spool = ctx.enter_context(tc.tile_pool(name="spool", bufs=1))
            pp = ctx.enter_context(tc.tile_pool(name="pp", bufs=2, space="PSUM"))
            dram = ctx.enter_context(tc.tile_pool(name="dram", bufs=1, space="DRAM"))

            # ---- constants / inputs to SBUF ----
            # chain-critical inputs FIRST on the sync ring (ahead of the W
            # stream); non-critical small ones on the scalar ring
            ident = cpool.tile([128, 128], f32)
            masks.make_identity(nc, ident[:])
            ones_row = cpool.tile([1, 128], f32)   # [1,P] lhsT for broadcasts
            nc.gpsimd.memset(ones_row[:], 1.0)

            pk_sb = cpool.tile([128, 65], f32)
            nc.sync.dma_start(pk_sb[:], pk_in[:])
            emb_sb = pk_sb[:, 0:HC]
            h0c_sb = pk_sb[:, HC : 2 * HC]
            h0own_sb = pk_sb[:, 2 * HC : 2 * HC + 1]
            bih_sb = pk_sb[:, 17:41]
            bhh_sb = pk_sb[:, 41:65]
            pr_sb = cpool.tile([1, 152], f32)
            nc.sync.dma_start(pr_sb[:], pr_in[:])
            attnb_sb = pr_sb[:, 0:MAX_LEN]
            combb_sb = pr_sb[:, MAX_LEN : MAX_LEN + 128]
            attnw_sb = cpool.tile([128, 16 * MAX_LEN], f32)
            nc.sync.dma_start(attnw_sb[:], attnw_in[:])
            enc_sb = cpool.tile([MAX_LEN, NHID], f32)
            nc.sync.dma_start(enc_sb[:], enc_in[:])
            combw_sb = cpool.tile([128, 16 * 128], f32)
            nc.sync.dma_start(combw_sb[:], combw_in[:])
            wih_sb = cpool.tile([128, 3 * NHID], f32)
            nc.sync.dma_start(wih_sb[:], wih_in[:])
            whh_sb = cpool.tile([128, 3 * NHID], f32)
            nc.sync.dma_start(whh_sb[:], whh_in[:])
            bout_sb = cpool.tile([1, VS], f32)
            nc.sync.dma_start(bout_sb[:], bout_in[:])


            # ---- attention (replicated) ----
            psA = pp.tile([128, MAX_LEN], f32, tag="psA", bufs=2)
            alog_ps = psA[0:1, 0:MAX_LEN]
            for c in range(HC):
                nc.tensor.matmul(
                    alog_ps,
                    emb_sb[:, c : c + 1],
                    attnw_sb[:, c * MAX_LEN : (c + 1) * MAX_LEN],
                    start=(c == 0),
                    stop=False,
                )
            for c in range(HC):
                nc.tensor.matmul(
                    alog_ps,
                    h0c_sb[:, c : c + 1],
                    attnw_sb[:, (HC + c) * MAX_LEN : (HC + c + 1) * MAX_LEN],
                    start=False,
                    stop=(c == HC - 1),
                )
            alog_sb = spool.tile([1, MAX_LEN], f32)
            nc.vector.tensor_add(alog_sb[:], alog_ps, attnb_sb[:])
            amax = spool.tile([1, 1], f32)
            nc.vector.reduce_max(amax[:], alog_sb[:], axis=mybir.AxisListType.X)
            namax = spool.tile([1, 1], f32)
            nc.vector.tensor_scalar_mul(namax[:], amax[:], -1.0)
            probs = spool.tile([1, MAX_LEN], f32)
            sume = spool.tile([1, 1], f32)
            nc.scalar.activation(
                probs[:], alog_sb[:], mybir.ActivationFunctionType.Exp,
                bias=namax[0:1, 0:1], accum_out=sume[:],
            )
            rinv = spool.tile([1, 1], f32)
            nc.vector.reciprocal(rinv[:], sume[:])
            attnp_sb = spool.tile([1, MAX_LEN], f32)
            nc.scalar.mul(attnp_sb[:], probs[:], rinv[0:1, 0:1])
            nc.scalar.dma_start(attn_out[:], attnp_sb[:])

            # transpose attn probs -> [24, 1]
            psS = pp.tile([128, 16], f32, tag="psS", bufs=1)
            nc.tensor.transpose(psS[0:MAX_LEN, 0:1], attnp_sb[:], ident[0:1, 0:1])
            awt_sb = spool.tile([MAX_LEN, 1], f32)
            nc.vector.tensor_copy(awt_sb[:], psS[0:MAX_LEN, 0:1])

            # attn_applied chunks: [128, HC]
            psA2 = pp.tile([128, MAX_LEN], f32, tag="psA", bufs=2)
            for c in range(HC):
                nc.tensor.matmul(
                    psA2[:, c : c + 1],
                    enc_sb[0:MAX_LEN, c * 128 : (c + 1) * 128],
                    awt_sb[:],
                    start=True,
                    stop=True,
                )
            aap_sb = spool.tile([128, HC], f32)
            nc.vector.tensor_copy(aap_sb[:], psA2[:, 0:HC])

            # ---- comb (sharded, thin-stationary): x row = relu(cat @ W_sh.T + b)
            # lhsT = cat chunk [128,1] (trivial weight load), rhs = W tile.
            psC = pp.tile([1, 512], f32, tag="psG", bufs=2)
            psC_ap = psC[0:1, 0:128]
            for c in range(HC):
                nc.tensor.matmul(
                    psC_ap, emb_sb[:, c : c + 1],
                    combw_sb[:, c * 128 : (c + 1) * 128],
                    start=(c == 0), stop=False)
            for c in range(HC):
                nc.tensor.matmul(
                    psC_ap, aap_sb[:, c : c + 1],
                    combw_sb[:, (HC + c) * 128 : (HC + c + 1) * 128],
                    start=False, stop=False)
            # bias via K=1 ones matmul, then relu
            nc.tensor.matmul(psC_ap, ones_row[0:1, 0:1], combb_sb[:],
                             start=False, stop=True)
            xrow_sb = spool.tile([1, 128], f32)
            nc.scalar.activation(
                xrow_sb[:], psC_ap, mybir.ActivationFunctionType.Relu)
            # transpose x row -> [128, 1] for use as stationary operand
            psX = pp.tile([128, 48], f32, tag="psA", bufs=2)
            nc.tensor.transpose(psX[:, 0:1], xrow_sb[:], ident[0:1, 0:1])
            xsh_sb = spool.tile([128, 1], f32)
            nc.vector.tensor_copy(xsh_sb[:], psX[:, 0:1])

            # ---- GRU, contraction-sharded with weights as the moving
            # operand: partial gi = x_chunk.T @ W_ih[:, j].T -> [1, 3072],
            # same for gh; AllReduce(add) of [1, 6144]; transpose-load to
            # [128, 48] chunk layout; elementwise GRU gives FULL h'
            # replicated on every core (no h all-gather).
            gpart_sb = spool.tile([1, 2 * 3 * NHID], f32)
            for n in range(6):
                psGI = pp.tile([1, 512], f32, tag="psG", bufs=2)
                nc.tensor.matmul(
                    psGI[:], xsh_sb[:], wih_sb[:, n * 512 : (n + 1) * 512],
                    start=True, stop=True)
                nc.vector.tensor_copy(gpart_sb[0:1, n * 512 : (n + 1) * 512], psGI[:])
            for n in range(6):
                psGH = pp.tile([1, 512], f32, tag="psG", bufs=2)
                nc.tensor.matmul(
                    psGH[:], h0own_sb[:], whh_sb[:, n * 512 : (n + 1) * 512],
                    start=True, stop=True)
                nc.vector.tensor_copy(
                    gpart_sb[0:1, 3 * NHID + n * 512 : 3 * NHID + (n + 1) * 512],
                    psGH[:])

            ar_in = dram.tile([1, 2 * 3 * NHID], f32)
            ar_out = dram.tile([1, 2 * 3 * NHID], f32, addr_space="Shared")
            nc.gpsimd.dma_start(ar_in[:], gpart_sb[:])
            nc.gpsimd.collective_compute(
                "AllReduce", mybir.AluOpType.add, replica_groups=RG,
                ins=[ar_in[:].opt()], outs=[ar_out[:].opt()],
            )
            # load as [48, 128] (contiguous rows) and transpose to [128, 48]
            g48_sb = spool.tile([48, 128], f32)
            nc.gpsimd.dma_start(g48_sb[:], ar_out[:].rearrange("o (a b) -> (o a) b", b=128))
            psG48 = pp.tile([128, 48], f32, tag="psA", bufs=2)
            nc.tensor.transpose(psG48[:], g48_sb[:], ident[0:48, 0:48])
            gf_sb = spool.tile([128, 48], f32)
            nc.vector.tensor_copy(gf_sb[:], psG48[:])
            # cols: 0:8 gi_r, 8:16 gi_z, 16:24 gi_n, 24:32 gh_r, 32:40 gh_z, 40:48 gh_n

            # elementwise GRU on [128, HC] chunk-layout tensors
            brz_sb = spool.tile([128, 2 * HC], f32)
            nc.vector.tensor_add(brz_sb[:], bih_sb[:, 0 : 2 * HC], bhh_sb[:, 0 : 2 * HC])
            rz0_sb = spool.tile([128, 2 * HC], f32)
            nc.vector.tensor_add(rz0_sb[:], gf_sb[:, 0 : 2 * HC], gf_sb[:, 3 * HC : 5 * HC])
            rzin_sb = spool.tile([128, 2 * HC], f32)
            nc.vector.tensor_add(rzin_sb[:], rz0_sb[:], brz_sb[:])
            rz_sb = spool.tile([128, 2 * HC], f32)
            nc.scalar.activation(
                rz_sb[:], rzin_sb[:], mybir.ActivationFunctionType.Sigmoid)
            hnb_sb = spool.tile([128, HC], f32)
            nc.vector.tensor_add(
                hnb_sb[:], gf_sb[:, 5 * HC : 6 * HC], bhh_sb[:, 2 * HC : 3 * HC])
            rhn_sb = spool.tile([128, HC], f32)
            nc.vector.tensor_mul(rhn_sb[:], rz_sb[:, 0:HC], hnb_sb[:])
            t1_sb = spool.tile([128, HC], f32)
            nc.vector.tensor_add(t1_sb[:], gf_sb[:, 2 * HC : 3 * HC], rhn_sb[:])
            t2_sb = spool.tile([128, HC], f32)
            nc.vector.tensor_add(t2_sb[:], t1_sb[:], bih_sb[:, 2 * HC : 3 * HC])
            n_sb = spool.tile([128, HC], f32)
            nc.scalar.activation(
                n_sb[:], t2_sb[:], mybir.ActivationFunctionType.Tanh)
            d_sb = spool.tile([128, HC], f32)
            nc.vector.tensor_sub(d_sb[:], h0c_sb[:], n_sb[:])
            zd_sb = spool.tile([128, HC], f32)
            nc.vector.tensor_mul(zd_sb[:], rz_sb[:, HC : 2 * HC], d_sb[:])
            hn_sb = spool.tile([128, HC], f32)
            nc.vector.tensor_add(hn_sb[:], n_sb[:], zd_sb[:])
            nc.scalar.dma_start(h_out[:], hn_sb[:])
            h_mm = cpool.tile([128, HC], wdt)
            nc.vector.tensor_copy(h_mm[:], hn_sb[:])

            # ---- out projection (streamed, vocab shard VS=6400, 16 tiles)
            # bias folded into the matmul accumulation via a K=1 ones matmul;
            # per-tile online softmax stats on partition 0 overlap the stream.
            logits_sb = cpool.tile([16, TN], f32)
            mrow = spool.tile([1, NT], f32)   # holds -m_t per tile
            zrow = spool.tile([1, NT], f32)
            for wc in range(NT // WG):
                w_tile = wpool.tile([128, WG * HC * TN], wdt, tag="wtile")
                nc.sync.dma_start(w_tile[:], wout_in[wc])
                for ti in range(WG):
                    t = wc * WG + ti
                    psT = pp.tile([1, TN], f32, tag="psT", bufs=3)
                    for k in range(HC):
                        nc.tensor.matmul(
                            psT[:],
                            h_mm[:, k : k + 1],
                            w_tile[:, (ti * HC + k) * TN : (ti * HC + k + 1) * TN],
                            start=(k == 0),
                            stop=False,
                        )
                    nc.tensor.matmul(
                        psT[:], ones_row[0:1, 0:1],
                        bout_sb[0:1, t * TN : (t + 1) * TN],
                        start=False, stop=True)
                    # compute engines can't address partition t directly (32-part
                    # alignment) — stage on partition 0, DMA-scatter to row t
                    lrow = spool.tile([1, TN], f32, tag="lrow", bufs=4)
                    nc.vector.tensor_copy(lrow[:], psT[:])
                    nc.scalar.dma_start(logits_sb[t : t + 1, :], lrow[:])
                    # online per-tile stats (mrow holds -max directly)
                    nc.vector.reduce_max(
                        mrow[0:1, t : t + 1], lrow[:], axis=mybir.AxisListType.X,
                        negate=True)
                    e_scr = spool.tile([1, TN], f32, tag="escr", bufs=2)
                    nc.scalar.activation(
                        e_scr[:], lrow[:], mybir.ActivationFunctionType.Exp,
                        bias=mrow[0:1, t : t + 1],
                        accum_out=zrow[0:1, t : t + 1])

            # ---- combine the 16 per-tile stats (partition 0) ----
            nmloc = spool.tile([1, 1], f32)
            nc.vector.tensor_reduce(
                nmloc[:], mrow[:], op=mybir.AluOpType.min,
                axis=mybir.AxisListType.X)
            et16 = spool.tile([1, NT], f32)
            nc.scalar.activation(
                et16[:], mrow[:], mybir.ActivationFunctionType.Exp,
                scale=-1.0, bias=nmloc[0:1, 0:1])
            sz16 = spool.tile([1, NT], f32)
            nc.vector.tensor_mul(sz16[:], et16[:], zrow[:])
            stats_sb = spool.tile([1, 2], f32)
            nc.vector.tensor_scalar_mul(stats_sb[0:1, 0:1], nmloc[:], -1.0)
            nc.vector.reduce_sum(
                stats_sb[0:1, 1:2], sz16[:], axis=mybir.AxisListType.X)

            # ---- AllGather stats ----
            st_in = dram.tile([1, 2], f32)
            st_out = dram.tile([NCORES, 2], f32, addr_space="Shared")
            nc.gpsimd.dma_start(st_in[:], stats_sb[:])
            nc.gpsimd.collective_compute(
                "AllGather", mybir.AluOpType.bypass, replica_groups=RG,
                ins=[st_in[:].opt()], outs=[st_out[:].opt()],
            )
            s16_sb = spool.tile([1, 2 * NCORES], f32)
            nc.gpsimd.dma_start(
                s16_sb[:], st_out[:].rearrange("(o a) b -> o (a b)", o=1))

            # global max M, then C = M + ln(sum_c Z_c exp(m_c - M))
            s16v = s16_sb[:].rearrange("p (a b) -> p a b", b=2)
            gM = spool.tile([1, 1], f32)
            nc.vector.reduce_max(gM[:], s16v[:, :, 0:1], axis=mybir.AxisListType.XY)
            ngM = spool.tile([1, 1], f32)
            nc.vector.tensor_scalar_mul(ngM[:], gM[:], -1.0)
            e8 = spool.tile([1, NCORES], f32)
            nc.scalar.activation(
                e8[:], s16v[:, :, 0:1], mybir.ActivationFunctionType.Exp,
                bias=ngM[0:1, 0:1])
            s8p = spool.tile([1, NCORES], f32)
            nc.vector.tensor_mul(s8p[:], e8[:], s16v[:, :, 1:2])
            Zg = spool.tile([1, 1], f32)
            nc.vector.reduce_sum(Zg[:], s8p[:], axis=mybir.AxisListType.X)
            lnZ = spool.tile([1, 1], f32)
            nc.scalar.activation(lnZ[:], Zg[:], mybir.ActivationFunctionType.Ln)
            C = spool.tile([1, 1], f32)
            nc.scalar.activation(
                C[:], lnZ[:], mybir.ActivationFunctionType.Identity,
                bias=gM[0:1, 0:1])
            nC = spool.tile([1, 1], f32)
            nc.vector.tensor_scalar_mul(nC[:], C[:], -1.0)
            psB3 = pp.tile([16, 1], f32, tag="psS", bufs=1)
            nc.tensor.matmul(psB3[:], ones_row[0:1, 0:16], nC[:], start=True, stop=True)
            nC16_sb = spool.tile([16, 1], f32)
            nc.vector.tensor_copy(nC16_sb[:], psB3[:])
            logp_sb = spool.tile([16, TN], f32)
            nc.scalar.activation(
                logp_sb[:], logits_sb[:], mybir.ActivationFunctionType.Identity,
                bias=nC16_sb[:, 0:1])
            nc.sync.dma_start(logp_out[:], logp_sb[:])

    nc.compile()
    return nc


def _prep_inputs(inp, hidden, encoder_outputs, emb_W, attn_W, attn_b,
                 comb_W, comb_b, W_ih, W_hh, b_ih, b_hh, out_W, out_b):
    """Shard/layout the full inputs into 8 per-core input maps."""
    f = np.float32
    idx = int(np.asarray(inp).ravel()[0])
    emb_row = np.asarray(emb_W[idx], dtype=f)                 # [1024]
    h0 = np.asarray(hidden, dtype=f).ravel()                  # [1024]
    enc = np.ascontiguousarray(np.asarray(encoder_outputs, dtype=f))  # [24,1024]

    def chunked_vec(v):
        # [1024] -> [128, 8] with [p, c] = v[c*128+p]
        return np.ascontiguousarray(v.reshape(HC, 128).T)

    emb_c = chunked_vec(emb_row)
    h0_c = chunked_vec(h0)

    # attn_W [24, 2048] -> [128, 16*24]
    aT = np.asarray(attn_W, dtype=f).T.reshape(16, 128, MAX_LEN)
    attnw = np.ascontiguousarray(aT.transpose(1, 0, 2).reshape(128, 16 * MAX_LEN))
    attnb = np.ascontiguousarray(np.asarray(attn_b, dtype=f).reshape(1, MAX_LEN))

    comb_W = np.asarray(comb_W, dtype=f)
    comb_b_a = np.asarray(comb_b, dtype=f)
    W_ih_a = np.asarray(W_ih, dtype=f)
    W_hh_a = np.asarray(W_hh, dtype=f)
    b_ih_a = np.asarray(b_ih, dtype=f)
    b_hh_a = np.asarray(b_hh, dtype=f)
    out_W_a = np.asarray(out_W, dtype=f)
    out_b_a = np.asarray(out_b, dtype=f)

    if WOUT_BF16:
        import ml_dtypes

        wout_dt = ml_dtypes.bfloat16
    else:
        wout_dt = f

    in_maps = []
    for j in range(NCORES):
        sl = slice(j * 128, (j + 1) * 128)
        # comb shard [128, 2048] -> [128(p), 16*128]
        cw = comb_W[sl]                                    # [128, 2048]
        cwT = cw.T.reshape(16, 128, 128)                   # [c, p, m]
        combw = np.ascontiguousarray(cwT.transpose(1, 0, 2).reshape(128, 16 * 128))
        combb = np.ascontiguousarray(comb_b_a[sl].reshape(1, 128))

        def gate_pack(W):
            # moving-operand layout: rhs[p, q] = W[q, j*128+p]
            return np.ascontiguousarray(W[:, j * 128 : (j + 1) * 128].T)

        wih = gate_pack(W_ih_a)
        whh = gate_pack(W_hh_a)

        def bias_pack(b):
            # [128, 3*HC]: col g*HC+c holds b[g*1024 + c*128 + p]
            return np.ascontiguousarray(
                b.reshape(3, HC, 128).transpose(2, 0, 1).reshape(128, 3 * HC))

        bih = bias_pack(b_ih_a)
        bhh = bias_pack(b_hh_a)

        # out_W vocab shard [VS, 1024] (zero-padded), bias shard with PAD_BIAS
        lo, hi = j * VS, min((j + 1) * VS, NOUT)
        nreal = max(0, hi - lo)
        wsh = np.zeros((VS, NHID), dtype=f)
        bsh = np.full((VS,), PAD_BIAS, dtype=f)
        if nreal > 0:
            wsh[:nreal] = out_W_a[lo:hi]
            bsh[:nreal] = out_b_a[lo:hi]
        WT = wsh.T                                         # [1024, 6400]
        # [NT, 128, HC*TN]: [t, p, k*TN+n] = WT[k*128+p, t*TN+n], grouped by WG
        warr = (
            WT.reshape(HC, 128, NT, TN).transpose(2, 1, 0, 3)
            .reshape(NT // WG, WG, 128, HC * TN).transpose(0, 2, 1, 3)
            .reshape(NT // WG, 128, WG * HC * TN)
        )
        warr = np.ascontiguousarray(warr.astype(wout_dt))
        barr = np.ascontiguousarray(bsh.reshape(1, VS))

        pk = np.concatenate(
            [emb_c, h0_c, h0[sl].reshape(128, 1), bih, bhh], axis=1)
        pr = np.concatenate([attnb.reshape(1, -1), combb.reshape(1, -1)], axis=1)
        in_maps.append({
            "pk_in": np.ascontiguousarray(pk),
            "pr_in": np.ascontiguousarray(pr),
            "enc_in": enc, "attnw_in": attnw,
            "combw_in": combw,
            "wih_in": wih, "whh_in": whh,
            "wout_in": warr, "bout_in": barr,
        })
    return in_maps


def run(trace=False, **inputs):
    from concourse.bass_utils import run_bass_kernel_spmd

    if "nc" not in _CACHE:
        _CACHE["nc"] = _build()
    nc = _CACHE["nc"]

    inputs.pop("encoder_output", None)  # unused by the reference computation
    in_maps = _prep_inputs(**inputs)
    res = run_bass_kernel_spmd(
        nc, in_maps, core_ids=list(range(NCORES)), trace=trace
    )

    logp = np.concatenate(
        [res.results[j]["logp_out"].reshape(-1) for j in range(NCORES)]
    )[:NOUT].reshape(1, NOUT).astype(np.float32)
    # h_out is [128, HC] chunk layout, full h' replicated on every core
    h = res.results[0]["h_out"].T.reshape(1, 1, NHID).astype(np.float32)
    attn = res.results[0]["attn_out"].reshape(1, MAX_LEN).astype(np.float32)
    return (logp, h, attn), res


def kernel(**inputs):
    out, _ = run(trace=bool(os.environ.get("KERNEL_TRACE")), **inputs)
    return out


# revision 24
# speedup vs baseline: 2.7978x; 1.3128x over previous
"""AttnDecoderRNN step on 8 TRN2 NeuronCores (Bass/Tile).

Sharding (per sharding hint): vocab-parallel out projection (embedding row
handled as a host-side gather/shard selection), hidden-sharded comb/GRU
matmuls, replicated attention. Collectives: AllGather of x (post-comb relu),
AllGather of h', AllGather of per-core log-softmax stats (max, sumexp).

Engine assignment (avoids the gpsimd/SWDGE backlog that delayed collective
triggers): gpsimd carries ONLY collective bounces + triggers + gathered
loads; small inputs ride the scalar HWDGE ring; big weights go first on the
sync ring ahead of the streamed out_W tiles.

Shapes: NHID=1024, NOUT=50257, MAX_LEN=24, batch=1.
Per-core vocab shard: VS=6400 (8*6400=51200 >= 50257; padding gets bias -1e4).
"""
import os
import sys
import types
import contextlib
import ctypes

import numpy as np

# ---------------------------------------------------------------------------
# antenv.axon_hooks shim: the container's antenv stub lacks this module, but
# concourse.bass_utils imports it when tracing is requested (BASS_TRACE=1).
# Provide it, with the ctypes NTFF profile hook libaxon exposes.
# ---------------------------------------------------------------------------
_HOOK = [None]


def _install_axon_hook_shim():
    if "antenv.axon_hooks" not in sys.modules:
        mod = types.ModuleType("antenv.axon_hooks")

        def set_axon_ntff_profile_hook(h):
            _HOOK[0] = h

        def get_axon_ntff_profile_hook():
            return _HOOK[0]

        mod.set_axon_ntff_profile_hook = set_axon_ntff_profile_hook
        mod.get_axon_ntff_profile_hook = get_axon_ntff_profile_hook
        sys.modules["antenv.axon_hooks"] = mod
        try:
            import antenv

            antenv.axon_hooks = mod
        except ImportError:
            pass
    if _HOOK[0] is None:
        so_path = "/opt/axon/libaxon_pjrt.so"
        try:
            lib = ctypes.CDLL(so_path)
        except OSError:
            return
        if not hasattr(lib, "axon_start_nrt_profile"):
            return
        lib.axon_start_nrt_profile.argtypes = [
            ctypes.POINTER(ctypes.c_int64),
            ctypes.c_size_t,
        ]
        lib.axon_start_nrt_profile.restype = ctypes.c_int64
        lib.axon_stop_nrt_profile.argtypes = [ctypes.c_char_p]
        lib.axon_stop_nrt_profile.restype = ctypes.c_int64

        @contextlib.contextmanager
        def _hook(output_dir, device_ids):
            import jax

            jax.devices()
            if device_ids:
                ids = (ctypes.c_int64 * len(device_ids))(*device_ids)
                rc = lib.axon_start_nrt_profile(ids, len(device_ids))
            else:
                rc = lib.axon_start_nrt_profile(None, 0)
            if rc != 0:
                raise RuntimeError(f"axon_start_nrt_profile rc={rc}")
            try:
                yield
            finally:
                n = lib.axon_stop_nrt_profile(str(output_dir).encode())
                print(f"profile: {n} file(s) -> {output_dir}", file=sys.stderr)

        sys.modules["antenv.axon_hooks"].set_axon_ntff_profile_hook(_hook)


_install_axon_hook_shim()

NCORES = 8
NHID = 1024
NOUT = 50257
MAX_LEN = 24
HC = NHID // 128          # 8 hidden chunks of 128
VS = 6400                 # vocab rows per core (padded)
TN = 400                  # out-projection free-dim tile (PSUM bank limit: 512 f32)
NT = VS // TN             # 16 logical tiles per core
WG = 4                    # logical tiles per W DMA chunk
PAD_BIAS = -1.0e4         # bias on padded vocab rows: exp() underflows to 0
WOUT_BF16 = os.environ.get("WOUT_DTYPE", "bf16") == "bf16"

_CACHE = {}


def _build():
    import concourse.bass as bass
    import concourse.tile as tile
    from concourse import bacc, mybir, masks
    from contextlib import ExitStack

    f32 = mybir.dt.float32
    wdt = mybir.dt.bfloat16 if WOUT_BF16 else f32

    nc = bacc.Bacc(
        "TRN2",
        target_bir_lowering=False,
        debug=False,
        enable_asserts=True,
        num_devices=NCORES,
    )

    # ---- I/O ----
    pk_in = nc.dram_tensor("pk_in", [128, 65], f32, kind="ExternalInput")
    pr_in = nc.dram_tensor("pr_in", [1, 152], f32, kind="ExternalInput")
    enc_in = nc.dram_tensor("enc_in", [MAX_LEN, NHID], f32, kind="ExternalInput")
    attnw_in = nc.dram_tensor("attnw_in", [128, 16 * MAX_LEN], f32, kind="ExternalInput")
    combw_in = nc.dram_tensor("combw_in", [128, 16 * 128], f32, kind="ExternalInput")
    wih_in = nc.dram_tensor("wih_in", [128, 3 * NHID], f32, kind="ExternalInput")
    whh_in = nc.dram_tensor("whh_in", [128, 3 * NHID], f32, kind="ExternalInput")
    wout_in = nc.dram_tensor(
        "wout_in", [NT // WG, 128, WG * HC * TN], wdt, kind="ExternalInput"
    )
    bout_in = nc.dram_tensor("bout_in", [1, VS], f32, kind="ExternalInput")

    logp_out = nc.dram_tensor("logp_out", [16, TN], f32, kind="ExternalOutput")
    h_out = nc.dram_tensor("h_out", [128, HC], f32, kind="ExternalOutput")
    attn_out = nc.dram_tensor("attn_out", [1, MAX_LEN], f32, kind="ExternalOutput")

    RG = [list(range(NCORES))]

    with tile.TileContext(nc) as tc:
        with ExitStack() as ctx:
            wpool = ctx.enter_context(tc.tile_pool(name="wpool", bufs=3))
            cpool = ctx.enter_context(tc.tile_pool(name="cpool", bufs=1))
            spool = ctx.enter_context(tc.tile_pool(name="spool", bufs=1))
            pp = ctx.enter_context(tc.tile_pool(name="pp", bufs=2, space="PSUM"))
            dram = ctx.enter_context(tc.tile_pool(name="dram", bufs=1, space="DRAM"))

            # ---- constants / inputs to SBUF ----
            # chain-critical inputs FIRST on the sync ring (ahead of the W
            # stream); non-critical small ones on the scalar ring
            ident = cpool.tile([128, 128], f32)
            masks.make_identity(nc, ident[:])
            ones_row = cpool.tile([1, 128], f32)   # [1,P] lhsT for broadcasts
            nc.gpsimd.memset(ones_row[:], 1.0)

            pk_sb = cpool.tile([128, 65], f32)
            nc.sync.dma_start(pk_sb[:], pk_in[:])
            emb_sb = pk_sb[:, 0:HC]
            h0c_sb = pk_sb[:, HC : 2 * HC]
            h0own_sb = pk_sb[:, 2 * HC : 2 * HC + 1]
            bih_sb = pk_sb[:, 17:41]
            bhh_sb = pk_sb[:, 41:65]
            pr_sb = cpool.tile([1, 152], f32)
            nc.sync.dma_start(pr_sb[:], pr_in[:])
            attnb_sb = pr_sb[:, 0:MAX_LEN]
            combb_sb = pr_sb[:, MAX_LEN : MAX_LEN + 128]
            attnw_sb = cpool.tile([128, 16 * MAX_LEN], f32)
            nc.sync.dma_start(attnw_sb[:], attnw_in[:])
            enc_sb = cpool.tile([MAX_LEN, NHID], f32)
            nc.sync.dma_start(enc_sb[:], enc_in[:])
            combw_sb = cpool.tile([128, 16 * 128], f32)
            nc.sync.dma_start(combw_sb[:], combw_in[:])
            wih_sb = cpool.tile([128, 3 * NHID], f32)
            nc.sync.dma_start(wih_sb[:], wih_in[:])
            whh_sb = cpool.tile([128, 3 * NHID], f32)
            nc.sync.dma_start(whh_sb[:], whh_in[:])
            bout_sb = cpool.tile([1, VS], f32)
            nc.sync.dma_start(bout_sb[:], bout_in[:])


            # ---- attention (replicated) ----
            psA = pp.tile([128, MAX_LEN], f32, tag="psA", bufs=2)
            alog_ps = psA[0:1, 0:MAX_LEN]
            for c in range(HC):
                nc.tensor.matmul(
                    alog_ps,
                    emb_sb[:, c : c + 1],
                    attnw_sb[:, c * MAX_LEN : (c + 1) * MAX_LEN],
                    start=(c == 0),
                    stop=False,
                )
            for c in range(HC):
                nc.tensor.matmul(
                    alog_ps,
                    h0c_sb[:, c : c + 1],
                    attnw_sb[:, (HC + c) * MAX_LEN : (HC + c + 1) * MAX_LEN],
                    start=False,
                    stop=(c == HC - 1),
                )
            alog_sb = spool.tile([1, MAX_LEN], f32)
            nc.vector.tensor_add(alog_sb[:], alog_ps, attnb_sb[:])
            amax = spool.tile([1, 1], f32)
            nc.vector.reduce_max(amax[:], alog_sb[:], axis=mybir.AxisListType.X)
            namax = spool.tile([1, 1], f32)
            nc.vector.tensor_scalar_mul(namax[:], amax[:], -1.0)
            probs = spool.tile([1, MAX_LEN], f32)
            sume = spool.tile([1, 1], f32)
            nc.scalar.activation(
                probs[:], alog_sb[:], mybir.ActivationFunctionType.Exp,
                bias=namax[0:1, 0:1], accum_out=sume[:],
            )
            rinv = spool.tile([1, 1], f32)
            nc.vector.reciprocal(rinv[:], sume[:])
            attnp_sb = spool.tile([1, MAX_LEN], f32)
            nc.scalar.mul(attnp_sb[:], probs[:], rinv[0:1, 0:1])
            nc.scalar.dma_start(attn_out[:], attnp_sb[:])

            # transpose attn probs -> [24, 1]
            psS = pp.tile([128, 16], f32, tag="psS", bufs=1)
            nc.tensor.transpose(psS[0:MAX_LEN, 0:1], attnp_sb[:], ident[0:1, 0:1])
            awt_sb = spool.tile([MAX_LEN, 1], f32)
            nc.vector.tensor_copy(awt_sb[:], psS[0:MAX_LEN, 0:1])

            # attn_applied chunks: [128, HC]
            psA2 = pp.tile([128, MAX_LEN], f32, tag="psA", bufs=2)
            for c in range(HC):
                nc.tensor.matmul(
                    psA2[:, c : c + 1],
                    enc_sb[0:MAX_LEN, c * 128 : (c + 1) * 128],
                    awt_sb[:],
                    start=True,
                    stop=True,
                )
            aap_sb = spool.tile([128, HC], f32)
            nc.vector.tensor_copy(aap_sb[:], psA2[:, 0:HC])

            # ---- comb (sharded, thin-stationary): x row = relu(cat @ W_sh.T + b)
            # lhsT = cat chunk [128,1] (trivial weight load), rhs = W tile.
            psC = pp.tile([1, 512], f32, tag="psG", bufs=2)
            psC_ap = psC[0:1, 0:128]
            for c in range(HC):
                nc.tensor.matmul(
                    psC_ap, emb_sb[:, c : c + 1],
                    combw_sb[:, c * 128 : (c + 1) * 128],
                    start=(c == 0), stop=False)
            for c in range(HC):
                nc.tensor.matmul(
                    psC_ap, aap_sb[:, c : c + 1],
                    combw_sb[:, (HC + c) * 128 : (HC + c + 1) * 128],
                    start=False, stop=False)
            # bias via K=1 ones matmul, then relu
            nc.tensor.matmul(psC_ap, ones_row[0:1, 0:1], combb_sb[:],
                             start=False, stop=True)
            xrow_sb = spool.tile([1, 128], f32)
            nc.scalar.activation(
                xrow_sb[:], psC_ap, mybir.ActivationFunctionType.Relu)
            # transpose x row -> [128, 1] for use as stationary operand
            psX = pp.tile([128, 48], f32, tag="psA", bufs=2)
            nc.tensor.transpose(psX[:, 0:1], xrow_sb[:], ident[0:1, 0:1])
            xsh_sb = spool.tile([128, 1], f32)
            nc.vector.tensor_copy(xsh_sb[:], psX[:, 0:1])

            # ---- GRU, contraction-sharded with weights as the moving
            # operand: partial gi = x_chunk.T @ W_ih[:, j].T -> [1, 3072],
            # same for gh; AllReduce(add) of [1, 6144]; transpose-load to
            # [128, 48] chunk layout; elementwise GRU gives FULL h'
            # replicated on every core (no h all-gather).
            gpart_sb = spool.tile([1, 2 * 3 * NHID], f32)
            for n in range(6):
                psGI = pp.tile([1, 512], f32, tag="psG", bufs=2)
                nc.tensor.matmul(
                    psGI[:], xsh_sb[:], wih_sb[:, n * 512 : (n + 1) * 512],
                    start=True, stop=True)
                nc.vector.tensor_copy(gpart_sb[0:1, n * 512 : (n + 1) * 512], psGI[:])
            for n in range(6):
                psGH = pp.tile([1, 512], f32, tag="psG", bufs=2)
                nc.tensor.matmul(
                    psGH[:], h0own_sb[:], whh_sb[:, n * 512 : (n + 1) * 512],
                    start=True, stop=True)
                nc.vector.tensor_copy(
                    gpart_sb[0:1, 3 * NHID + n * 512 : 3 * NHID + (n + 1) * 512],
                    psGH[:])

            ar_in = dram.tile([1, 2 * 3 * NHID], f32)
            ar_out = dram.tile([1, 2 * 3 * NHID], f32, addr_space="Shared")
            nc.gpsimd.dma_start(ar_in[:], gpart_sb[:])
            nc.gpsimd.collective_compute(
                "AllReduce", mybir.AluOpType.add, replica_groups=RG,
                ins=[ar_in[:].opt()], outs=[ar_out[:].opt()],
            )
            # load as [48, 128] (contiguous rows) and transpose to [128, 48]
            g48_sb = spool.tile([48, 128], f32)
            nc.gpsimd.dma_start(g48_sb[:], ar_out[:].rearrange("o (a b) -> (o a) b", b=128))
            psG48 = pp.tile([128, 48], f32, tag="psA", bufs=2)
            nc.tensor.transpose(psG48[:], g48_sb[:], ident[0:48, 0:48])
            gf_sb = spool.tile([128, 48], f32)
            nc.vector.tensor_copy(gf_sb[:], psG48[:])
            # cols: 0:8 gi_r, 8:16 gi_z, 16:24 gi_n, 24:32 gh_r, 32:40 gh_z, 40:48 gh_n

            # elementwise GRU on [128, HC] chunk-layout tensors
            brz_sb = spool.tile([128, 2 * HC], f32)
            nc.vector.tensor_add(brz_sb[:], bih_sb[:, 0 : 2 * HC], bhh_sb[:, 0 : 2 * HC])
            rz0_sb = spool.tile([128, 2 * HC], f32)
            nc.vector.tensor_add(rz0_sb[:], gf_sb[:, 0 : 2 * HC], gf_sb[:, 3 * HC : 5 * HC])
            rzin_sb = spool.tile([128, 2 * HC], f32)
            nc.vector.tensor_add(rzin_sb[:], rz0_sb[:], brz_sb[:])
            rz_sb = spool.tile([128, 2 * HC], f32)
            nc.scalar.activation(
                rz_sb[:], rzin_sb[:], mybir.ActivationFunctionType.Sigmoid)
            hnb_sb = spool.tile([128, HC], f32)
            nc.vector.tensor_add(
                hnb_sb[:], gf_sb[:, 5 * HC : 6 * HC], bhh_sb[:, 2 * HC : 3 * HC])
            rhn_sb = spool.tile([128, HC], f32)
            nc.vector.tensor_mul(rhn_sb[:], rz_sb[:, 0:HC], hnb_sb[:])
            t1_sb = spool.tile([128, HC], f32)
            nc.vector.tensor_add(t1_sb[:], gf_sb[:, 2 * HC : 3 * HC], rhn_sb[:])
            t2_sb = spool.tile([128, HC], f32)
            nc.vector.tensor_add(t2_sb[:], t1_sb[:], bih_sb[:, 2 * HC : 3 * HC])
            n_sb = spool.tile([128, HC], f32)
            nc.scalar.activation(
                n_sb[:], t2_sb[:], mybir.ActivationFunctionType.Tanh)
            d_sb = spool.tile([128, HC], f32)
            nc.vector.tensor_sub(d_sb[:], h0c_sb[:], n_sb[:])
            zd_sb = spool.tile([128, HC], f32)
            nc.vector.tensor_mul(zd_sb[:], rz_sb[:, HC : 2 * HC], d_sb[:])
            hn_sb = spool.tile([128, HC], f32)
            nc.vector.tensor_add(hn_sb[:], n_sb[:], zd_sb[:])
            nc.scalar.dma_start(h_out[:], hn_sb[:])
            h_mm = cpool.tile([128, HC], wdt)
            nc.vector.tensor_copy(h_mm[:], hn_sb[:])

            # ---- out projection (streamed, vocab shard VS=6400, 16 tiles)
            # bias folded into the matmul accumulation via a K=1 ones matmul;
            # per-tile online softmax stats on partition 0 overlap the stream.
            logits_sb = cpool.tile([16, TN], f32)
            mrow = spool.tile([1, NT], f32)   # holds -m_t per tile
            zrow = spool.tile([1, NT], f32)
            for wc in range(NT // WG):
                w_tile = wpool.tile([128, WG * HC * TN], wdt, tag="wtile")
                nc.sync.dma_start(w_tile[:], wout_in[wc])
                # 4-way PE column-tiling: the 4 tiles of this chunk run in
                # disjoint 32-column strips of the array concurrently
                psT4 = pp.tile([128, TN], f32, tag="psT", bufs=2)
                for k in range(HC):
                    for q in range(WG):
                        nc.tensor.matmul(
                            psT4[32 * q : 32 * q + 1, :],
                            h_mm[:, k : k + 1],
                            w_tile[:, (q * HC + k) * TN : (q * HC + k + 1) * TN],
                            start=(k == 0),
                            stop=(k == HC - 1),
                            tile_position=(0, 32 * q),
                        )
                for q in range(WG):
                    t = wc * WG + q
                    # bias add fused into the PSUM drain; engines can read the
                    # 32-aligned psum rows directly
                    lrow = spool.tile([1, TN], f32, tag="lrow", bufs=4)
                    nc.vector.tensor_add(
                        lrow[:], psT4[32 * q : 32 * q + 1, :],
                        bout_sb[0:1, t * TN : (t + 1) * TN])
                    nc.scalar.dma_start(logits_sb[t : t + 1, :], lrow[:])
                    # online per-tile stats (mrow holds -max directly)
                    nc.vector.reduce_max(
                        mrow[0:1, t : t + 1], lrow[:], axis=mybir.AxisListType.X,
                        negate=True)
                    e_scr = spool.tile([1, TN], f32, tag="escr", bufs=2)
                    nc.scalar.activation(
                        e_scr[:], lrow[:], mybir.ActivationFunctionType.Exp,
                        bias=mrow[0:1, t : t + 1],
                        accum_out=zrow[0:1, t : t + 1])

            # ---- combine the 16 per-tile stats (partition 0) ----
            nmloc = spool.tile([1, 1], f32)
            nc.vector.tensor_reduce(
                nmloc[:], mrow[:], op=mybir.AluOpType.min,
                axis=mybir.AxisListType.X)
            et16 = spool.tile([1, NT], f32)
            nc.scalar.activation(
                et16[:], mrow[:], mybir.ActivationFunctionType.Exp,
                scale=-1.0, bias=nmloc[0:1, 0:1])
            sz16 = spool.tile([1, NT], f32)
            nc.vector.tensor_mul(sz16[:], et16[:], zrow[:])
            stats_sb = spool.tile([1, 2], f32)
            nc.vector.tensor_scalar_mul(stats_sb[0:1, 0:1], nmloc[:], -1.0)
            nc.vector.reduce_sum(
                stats_sb[0:1, 1:2], sz16[:], axis=mybir.AxisListType.X)

            # ---- AllGather stats ----
            st_in = dram.tile([1, 2], f32)
            st_out = dram.tile([NCORES, 2], f32, addr_space="Shared")
            nc.gpsimd.dma_start(st_in[:], stats_sb[:])
            nc.gpsimd.collective_compute(
                "AllGather", mybir.AluOpType.bypass, replica_groups=RG,
                ins=[st_in[:].opt()], outs=[st_out[:].opt()],
            )
            s16_sb = spool.tile([1, 2 * NCORES], f32)
            nc.gpsimd.dma_start(
                s16_sb[:], st_out[:].rearrange("(o a) b -> o (a b)", o=1))

            # global max M, then C = M + ln(sum_c Z_c exp(m_c - M))
            s16v = s16_sb[:].rearrange("p (a b) -> p a b", b=2)
            gM = spool.tile([1, 1], f32)
            nc.vector.reduce_max(gM[:], s16v[:, :, 0:1], axis=mybir.AxisListType.XY)
            ngM = spool.tile([1, 1], f32)
            nc.vector.tensor_scalar_mul(ngM[:], gM[:], -1.0)
            e8 = spool.tile([1, NCORES], f32)
            nc.scalar.activation(
                e8[:], s16v[:, :, 0:1], mybir.ActivationFunctionType.Exp,
                bias=ngM[0:1, 0:1])
            s8p = spool.tile([1, NCORES], f32)
            nc.vector.tensor_mul(s8p[:], e8[:], s16v[:, :, 1:2])
            Zg = spool.tile([1, 1], f32)
            nc.vector.reduce_sum(Zg[:], s8p[:], axis=mybir.AxisListType.X)
            lnZ = spool.tile([1, 1], f32)
            nc.scalar.activation(lnZ[:], Zg[:], mybir.ActivationFunctionType.Ln)
            C = spool.tile([1, 1], f32)
            nc.scalar.activation(
                C[:], lnZ[:], mybir.ActivationFunctionType.Identity,
                bias=gM[0:1, 0:1])
            nC = spool.tile([1, 1], f32)
            nc.vector.tensor_scalar_mul(nC[:], C[:], -1.0)
            psB3 = pp.tile([16, 1], f32, tag="psS", bufs=1)
            nc.tensor.matmul(psB3[:], ones_row[0:1, 0:16], nC[:], start=True, stop=True)
            nC16_sb = spool.tile([16, 1], f32)
            nc.vector.tensor_copy(nC16_sb[:], psB3[:])
            logp_sb = spool.tile([16, TN], f32)
            nc.scalar.activation(
                logp_sb[:], logits_sb[:], mybir.ActivationFunctionType.Identity,
                bias=nC16_sb[:, 0:1])
            nc.sync.dma_start(logp_out[:], logp_sb[:])

    nc.compile()
    return nc


def _prep_inputs(inp, hidden, encoder_outputs, emb_W, attn_W, attn_b,
                 comb_W, comb_b, W_ih, W_hh, b_ih, b_hh, out_W, out_b):
    """Shard/layout the full inputs into 8 per-core input maps."""
    f = np.float32
    idx = int(np.asarray(inp).ravel()[0])
    emb_row = np.asarray(emb_W[idx], dtype=f)                 # [1024]
    h0 = np.asarray(hidden, dtype=f).ravel()                  # [1024]
    enc = np.ascontiguousarray(np.asarray(encoder_outputs, dtype=f))  # [24,1024]

    def chunked_vec(v):
        # [1024] -> [128, 8] with [p, c] = v[c*128+p]
        return np.ascontiguousarray(v.reshape(HC, 128).T)

    emb_c = chunked_vec(emb_row)
    h0_c = chunked_vec(h0)

    # attn_W [24, 2048] -> [128, 16*24]
    aT = np.asarray(attn_W, dtype=f).T.reshape(16, 128, MAX_LEN)
    attnw = np.ascontiguousarray(aT.transpose(1, 0, 2).reshape(128, 16 * MAX_LEN))
    attnb = np.ascontiguousarray(np.asarray(attn_b, dtype=f).reshape(1, MAX_LEN))

    comb_W = np.asarray(comb_W, dtype=f)
    comb_b_a = np.asarray(comb_b, dtype=f)
    W_ih_a = np.asarray(W_ih, dtype=f)
    W_hh_a = np.asarray(W_hh, dtype=f)
    b_ih_a = np.asarray(b_ih, dtype=f)
    b_hh_a = np.asarray(b_hh, dtype=f)
    out_W_a = np.asarray(out_W, dtype=f)
    out_b_a = np.asarray(out_b, dtype=f)

    if WOUT_BF16:
        import ml_dtypes

        wout_dt = ml_dtypes.bfloat16
    else:
        wout_dt = f

    in_maps = []
    for j in range(NCORES):
        sl = slice(j * 128, (j + 1) * 128)
        # comb shard [128, 2048] -> [128(p), 16*128]
        cw = comb_W[sl]                                    # [128, 2048]
        cwT = cw.T.reshape(16, 128, 128)                   # [c, p, m]
        combw = np.ascontiguousarray(cwT.transpose(1, 0, 2).reshape(128, 16 * 128))
        combb = np.ascontiguousarray(comb_b_a[sl].reshape(1, 128))

        def gate_pack(W):
            # moving-operand layout: rhs[p, q] = W[q, j*128+p]
            return np.ascontiguousarray(W[:, j * 128 : (j + 1) * 128].T)

        wih = gate_pack(W_ih_a)
        whh = gate_pack(W_hh_a)

        def bias_pack(b):
            # [128, 3*HC]: col g*HC+c holds b[g*1024 + c*128 + p]
            return np.ascontiguousarray(
                b.reshape(3, HC, 128).transpose(2, 0, 1).reshape(128, 3 * HC))

        bih = bias_pack(b_ih_a)
        bhh = bias_pack(b_hh_a)

        # out_W vocab shard [VS, 1024] (zero-padded), bias shard with PAD_BIAS
        lo, hi = j * VS, min((j + 1) * VS, NOUT)
        nreal = max(0, hi - lo)
        wsh = np.zeros((VS, NHID), dtype=f)
        bsh = np.full((VS,), PAD_BIAS, dtype=f)
        if nreal > 0:
            wsh[:nreal] = out_W_a[lo:hi]
            bsh[:nreal] = out_b_a[lo:hi]
        WT = wsh.T                                         # [1024, 6400]
        # [NT, 128, HC*TN]: [t, p, k*TN+n] = WT[k*128+p, t*TN+n], grouped by WG
        warr = (
            WT.reshape(HC, 128, NT, TN).transpose(2, 1, 0, 3)
            .reshape(NT // WG, WG, 128, HC * TN).transpose(0, 2, 1, 3)
            .reshape(NT // WG, 128, WG * HC * TN)
        )
        warr = np.ascontiguousarray(warr.astype(wout_dt))
        barr = np.ascontiguousarray(bsh.reshape(1, VS))

        pk = np.concatenate(
            [emb_c, h0_c, h0[sl].reshape(128, 1), bih, bhh], axis=1)
        pr = np.concatenate([attnb.reshape(1, -1), combb.reshape(1, -1)], axis=1)
        in_maps.append({
            "pk_in": np.ascontiguousarray(pk),
            "pr_in": np.ascontiguousarray(pr),
            "enc_in": enc, "attnw_in": attnw,
            "combw_in": combw,
            "wih_in": wih, "whh_in": whh,
            "wout_in": warr, "bout_in": barr,
        })
    return in_maps


def run(trace=False, **inputs):
    from concourse.bass_utils import run_bass_kernel_spmd

    if "nc" not in _CACHE:
        _CACHE["nc"] = _build()
    nc = _CACHE["nc"]

    inputs.pop("encoder_output", None)  # unused by the reference computation
    in_maps = _prep_inputs(**inputs)
    res = run_bass_kernel_spmd(
        nc, in_maps, core_ids=list(range(NCORES)), trace=trace
    )

    logp = np.concatenate(
        [res.results[j]["logp_out"].reshape(-1) for j in range(NCORES)]
    )[:NOUT].reshape(1, NOUT).astype(np.float32)
    # h_out is [128, HC] chunk layout, full h' replicated on every core
    h = res.results[0]["h_out"].T.reshape(1, 1, NHID).astype(np.float32)
    attn = res.results[0]["attn_out"].reshape(1, MAX_LEN).astype(np.float32)
    return (logp, h, attn), res


def kernel(**inputs):
    out, _ = run(trace=bool(os.environ.get("KERNEL_TRACE")), **inputs)
    return out
